# revision 17
# baseline (speedup 1.0000x reference)
"""Multi-head self-attention TRN2 Bass kernel.

Problem: x[4,2048,512], 8 heads of d=64, scale 1/sqrt(512) (full feature dim).

Sharding: 8 cores = (batch b in 0..3) x (head-group hg in 0..1). Each core
handles one batch element and 4 heads (256 of the 512 features), computing a
partial output projection z_partial = attn_heads @ Wo[hg rows].  The host
sums the two partials per batch and adds (bv @ Wo + bo).

Engine split (PE is the bottleneck; exp is split across ACT and DVE):
  - S^T tiles [128 j, 1024 i] on PE (f32r, two heads packed via
    tile_position), PV accumulation on PE with a ones-column rowsum.
  - exp: head 0 tiles via ScalarE AF.Exp; head 1 tiles via a custom DVE op
    EXP_POLY16_ANT computing ((x*a2+a1)*x+a0)^16 (deg-2 minimax of
    exp(x*SCALE/16), then 4 squarings; one 8-stage DVE instruction).
  - O_norm [128 i, 128 d] -> O^T transposes via DMA xbar transpose
    (dma_start_transpose), straight into the ot[k] staging layout.
  - z chunks: PE matmul -> ACT copy -> DMA out (biases folded to host).
  - V-ones memsets on GPSIMD (SBUF only); V copies + normalize split DVE/ACT.

Scheduling: per jt, S(jt+1) is emitted before PV(jt); background FIFO items
(projections, z chunks) pop once per jt through the single spare PSUM bank.
"""

import sys
import os

sys.path.insert(0, "/opt/trn_rl_repo")

import numpy as np

B, N, F = 4, 2048, 512
H, D = 8, 64
P = 128
DH = 256   # features per core (4 heads)
NPAIR = 2  # head pairs per core
KT = F // P          # 4 k-tiles over input features
ICB = 1024           # i-chunk per round
NICB = N // ICB      # 2
NT = N // P          # 16 j-tiles
NSUB = ICB // P      # 8 i-subtiles per chunk
SCALE = 1.0 / float(np.float32(F) ** 0.5)

# deg-2 minimax of exp(y) on |y| <= 0.208, y = s*SCALE/16; exp = p^16.
# Fit in fit_poly.py: rel err 5.7e-4 -> 9.2e-3 after ^16 (worst case |S|=75).
EXPC = (1.0000015, 0.0027809313, 3.828529e-06)  # a0, a1*k, a2*k^2 folded

_cache = {}


def _bf_np():
    import ml_dtypes

    return np.dtype(ml_dtypes.bfloat16)


def _exp_op():
    """Register (once) and return the custom DVE poly-exp op."""
    if "exp_op" in _cache:
        return _cache["exp_op"]
    from concourse.dve_spec import Spec, Src0, C0, C1, C2, sq
    from concourse import dve_ops

    def ref(in0, in1, c0, c1, c2):
        x = in0.astype(np.float32)
        p = (x * c2 + c1) * x + c0
        for _ in range(4):
            p = p * p
        return p

    for op in dve_ops.OPS:
        if op.name == "EXP_POLY16_ANT":
            _cache["exp_op"] = op
            return op
    op = dve_ops.DveOp(
        "EXP_POLY16_ANT",
        Spec(body=sq(sq(sq(sq((Src0 * C2 + C1) * Src0 + C0)))), reference=ref),
        subdim=False,
        uops_sha={"v3": "571513505c6c4e44", "v4": "f56cfa9f44bcd3e4"},
    )
    dve_ops.OPS.append(op)
    dve_ops.CUSTOM_DVE_SPECS[op.name] = op.spec
    dve_ops._SUB_OPCODE_FOR_NAME[op.name] = max(
        dve_ops._SUB_OPCODE_FOR_NAME.values()) + 1
    _cache["exp_op"] = op
    return op


def build():
    """Build + bass-compile the per-core program."""
    import concourse.tile as tile
    from concourse import bacc, mybir
    from contextlib import ExitStack

    f32 = mybir.dt.float32
    f32r = mybir.dt.float32r
    bf = mybir.dt.bfloat16
    AF = mybir.ActivationFunctionType
    EXP_OP = _exp_op()

    n = N
    nc = bacc.Bacc("TRN2", target_bir_lowering=False, debug=False)

    xt_d = nc.dram_tensor("xtb", [F, n], bf, kind="ExternalInput").ap()
    wkqv_d = nc.dram_tensor("wkqv", [F, 3 * DH], bf, kind="ExternalInput").ap()
    wo_d = nc.dram_tensor("wob", [DH, F], bf, kind="ExternalInput").ap()
    # packed biases: cols 0-1 bk, 2-3 bq
    bias_d = nc.dram_tensor("bias4", [P, 4], f32, kind="ExternalInput").ap()
    id_d = nc.dram_tensor("ident", [P, P], bf, kind="ExternalInput").ap()
    zt_d = nc.dram_tensor("zt", [F, n], bf, kind="ExternalOutput").ap()

    def dve_exp(dst, src):
        nc.vector._custom_dve(EXP_OP, out=dst, in0=src,
                              s0=EXPC[0], s1=EXPC[1], imm2=EXPC[2])

    with tile.TileContext(nc) as tc, ExitStack() as ctx:
        const = ctx.enter_context(tc.tile_pool(name="const", bufs=1))
        pt_pool = ctx.enter_context(tc.tile_pool(name="pt", bufs=10))
        rc_pool = ctx.enter_context(tc.tile_pool(name="rc", bufs=8))
        on_pool = ctx.enter_context(tc.tile_pool(name="on", bufs=16))
        zs_pool = ctx.enter_context(tc.tile_pool(name="zs", bufs=4))
        ps_s = ctx.enter_context(tc.tile_pool(name="ps_s", bufs=4, space="PSUM"))
        po_pool = ctx.enter_context(tc.tile_pool(name="po", bufs=3, space="PSUM"))
        pz_pool = ctx.enter_context(tc.tile_pool(name="pz", bufs=1, space="PSUM"))

        # ---- DMA loads ------------------------------------------------------
        xt = [const.tile([P, n], bf, tag=f"xt{k}", name=f"xt{k}") for k in range(KT)]
        wkqv = [const.tile([P, 3 * DH], bf, tag=f"wkqv{k}", name=f"wkqv{k}")
                for k in range(KT)]
        wk = [wkqv[k][:, 0:DH] for k in range(KT)]
        wq = [wkqv[k][:, DH:2 * DH] for k in range(KT)]
        wv = [wkqv[k][:, 2 * DH:3 * DH] for k in range(KT)]
        bias_sb = const.tile([P, 4], f32, tag="bias4", name="bias_sb")
        bk_sb = bias_sb[:, 0:NPAIR]
        bq_sb = bias_sb[:, NPAIR:2 * NPAIR]
        # warm tiles first: a dummy matmul starts the PE p-state ramp at t~1us
        # so the prologue projections run at full clock; the exp warms the
        # ScalarE table while DMAs stream in
        warm = const.tile([1, 1], f32, tag="warm", name="warm")
        nc.gpsimd.memset(warm[:], 0.0)
        warmmm = pz_pool.tile([P, 512], f32, tag="pz", name="warmmm")
        nc.tensor.matmul(warmmm[0:1, 0:1], warm[:], warm[:],
                         start=True, stop=True)
        nc.scalar.activation(warm[:], warm[:], AF.Exp)

        # first wave: weights + the first 512 cols of each k-tile (enough for
        # the q/k half-0 projections and j-tiles 0-3) to minimize the
        # DMA-serialized latency to the first S tile
        nc.sync.dma_start(wkqv[0][:], wkqv_d[0:P, :])
        nc.sync.dma_start(xt[0][:, 0:512], xt_d[0:P, 0:512])
        nc.sync.dma_start(bias_sb[:], bias_d[:])
        for k in range(1, KT):
            nc.sync.dma_start(wkqv[k][:], wkqv_d[k * P:(k + 1) * P, :])
            nc.sync.dma_start(xt[k][:, 0:512], xt_d[k * P:(k + 1) * P, 0:512])
        for k in range(KT):
            nc.sync.dma_start(xt[k][:, 512:ICB], xt_d[k * P:(k + 1) * P, 512:ICB])
        for k in range(KT):
            nc.sync.dma_start(xt[k][:, 1024:n], xt_d[k * P:(k + 1) * P, 1024:n])
        wo = [const.tile([P, F], bf, tag=f"wo{k}", name=f"wo{k}") for k in range(DH // P)]
        for k in range(DH // P):
            nc.sync.dma_start(wo[k][:], wo_d[k * P:(k + 1) * P, :])
        ident = const.tile([P, P], bf, tag="ident", name="ident")
        nc.sync.dma_start(ident[:], id_d[:])

        # persistent activations
        qt = [const.tile([P, n], f32r, tag=f"qt{p}", name=f"qt{p}") for p in range(NPAIR)]
        kt_sb = [const.tile([P, n], f32r, tag=f"kt{p}", name=f"ktsb{p}") for p in range(NPAIR)]
        # V per j-tile: [128, 260] bf16, head hl at cols [65*hl, 65*hl+64),
        # ones at col 65*hl+64 (accumulates softmax row-sums in PV).
        v_sb = [const.tile([P, 4 * (D + 1)], bf, tag=f"v{j}", name=f"v{j}")
                for j in range(NT)]
        for j in range(NT):
            nc.gpsimd.memset(v_sb[j][:], 1.0)
        # O^T staging for the z projection: [128 d, n] bf16 per k-tile (pair)
        ot = [const.tile([P, n], bf, tag=f"ot{p}", name=f"ot{p}") for p in range(NPAIR)]

        def v4(ap):
            return ap.rearrange("p (h c) -> p h c", h=4)

        # ---- projections -----------------------------------------------------
        def proj_qk_half(p, w_t, b_sb, dst, ib, half, pool, bias_eng="vector"):
            """One 512-wide half of a Q/K projection chunk."""
            ps = pool.tile([P, 512], f32, tag="pz", name="pjh") if pool is pz_pool \
                else pool.tile([P, 512], f32, tag="st", name="pjs")
            psl = ps[:, 0:512]
            isl = slice(ib * ICB + half * 512, ib * ICB + (half + 1) * 512)
            for k in range(KT):
                nc.tensor.matmul(
                    psl,
                    w_t[k][:, p * P:(p + 1) * P],
                    xt[k][:, isl],
                    start=(k == 0),
                    stop=(k == KT - 1),
                )
            if bias_eng == "act":
                nc.scalar.activation(dst[p][:, isl], psl, AF.Identity,
                                     bias=b_sb[:, p:p + 1])
            else:
                nc.vector.tensor_scalar_add(dst[p][:, isl], psl, b_sb[:, p:p + 1])

        def proj_v_pair(j0, pool):
            """V for j-tiles j0, j0+1 sharing one pz bank (two 256-col slices)."""
            ps = pool.tile([P, 512], f32, tag="pz", name="pvh") if pool is pz_pool \
                else pool.tile([P, 512], f32, tag="st", name="pvs")
            for m, j in enumerate((j0, j0 + 1)):
                psl = ps[:, m * DH:(m + 1) * DH]
                for k in range(KT):
                    nc.tensor.matmul(
                        psl,
                        xt[k][:, j * P:(j + 1) * P],
                        wv[k][:],
                        start=(k == 0 and m == 0),
                        stop=(k == KT - 1 and m == 1),
                        skip_group_check=True,
                    )
            for m, j in enumerate((j0, j0 + 1)):
                psl = ps[:, m * DH:(m + 1) * DH]
                nc.vector.tensor_copy(v4(v_sb[j][:])[:, :, 0:D], v4(psl))

        zsb_by_ft = {}

        def z_chunk(icb, ft, ch, pool, copy_eng="act", dma_eng=None):
            """z^T[ft*128:(ft+1)*128, 512-chunk ch of icb]; both chunks of an
            (icb, ft) share one [128,1024] bf16 staging tile and one DMA."""
            if pool is pz_pool:
                zp = pool.tile([P, 512], f32, tag="pz", name="zp")
                zpl = zp[:, 0:512]
            else:
                zp = pool.tile([P, 512], f32, tag="st", name="zps")
                zpl = zp[:, 0:512]
            isl = slice(icb * ICB + ch * 512, icb * ICB + (ch + 1) * 512)
            for k in range(DH // P):
                nc.tensor.matmul(
                    zpl,
                    wo[k][:, ft * P:(ft + 1) * P],
                    ot[k][:, isl],
                    start=(k == 0),
                    stop=(k == DH // P - 1),
                )
            if (icb, ft) not in zsb_by_ft:
                zsb_by_ft[(icb, ft)] = zs_pool.tile([P, ICB], bf, tag="zt",
                                                    name="zsb")
            zsb = zsb_by_ft[(icb, ft)]
            if copy_eng == "act":
                nc.scalar.activation(zsb[:, ch * 512:(ch + 1) * 512], zpl, AF.Copy)
            else:
                nc.vector.tensor_copy(zsb[:, ch * 512:(ch + 1) * 512], zpl)
            if ch == 1:
                eng = {None: nc.sync, "act": nc.scalar}[dma_eng]
                eng.dma_start(
                    zt_d[ft * P:(ft + 1) * P, icb * ICB:(icb + 1) * ICB], zsb[:])

        # ---- attention rounds ------------------------------------------------
        def ob_slice(ob, h, sub):
            """PSUM accumulator slice [128, 65] for (head h, i-subtile sub)."""
            if sub < 7:
                t = ob[h]
                c0 = sub * (D + 1)
            else:
                t = ob[2]
                c0 = h * (D + 1)
            return t[:, c0:c0 + D + 1]

        def emit_S_half(p, icb, jt, h, half):
            sps = ps_s.tile([P, 512], f32, tag="st", name="sps")
            hp = slice(D * h, D * (h + 1))
            isl = slice(icb * ICB + half * 512, icb * ICB + (half + 1) * 512)
            nc.tensor.matmul(
                sps[:, 0:512],
                kt_sb[p][hp, jt * P:(jt + 1) * P],
                qt[p][hp, isl],
                start=True,
                stop=True,
                tile_position=(D * h, 0),
            )
            return sps

        def emit_exp(dst, src, eng):
            if eng == "act":
                nc.scalar.activation(dst, src, AF.Exp, scale=SCALE)
            else:
                dve_exp(dst, src)

        # exp routing: head 0 -> ACT, head 1 -> DVE (tunable per round)
        def exp_eng(r, jt, h):
            return "act" if h == 0 else "dve"

        # prologue: K/Q first chunks + jt0's four S halves + the FIRST EXPS.
        proj_qk_half(0, wq, bq_sb, qt, 0, 0, ps_s, bias_eng="act")
        proj_qk_half(0, wk, bk_sb, kt_sb, 0, 0, ps_s)
        s000 = emit_S_half(0, 0, 0, 0, 0)
        pt000 = pt_pool.tile([P, 512], bf, tag="pt", name="pt000")
        nc.scalar.activation(pt000[:], s000[:], AF.Exp, scale=SCALE)
        proj_qk_half(0, wq, bq_sb, qt, 0, 1, ps_s, bias_eng="act")
        s001 = emit_S_half(0, 0, 0, 0, 1)
        proj_qk_half(0, wk, bk_sb, kt_sb, 0, 1, pz_pool)
        pt001 = pt_pool.tile([P, 512], bf, tag="pt", name="pt001")
        nc.scalar.activation(pt001[:], s001[:], AF.Exp, scale=SCALE)
        s010 = emit_S_half(0, 0, 0, 1, 0)
        s011 = emit_S_half(0, 0, 0, 1, 1)
        pt010 = pt_pool.tile([P, 512], bf, tag="pt", name="pt010")
        dve_exp(pt010[:], s010[:])
        pt011 = pt_pool.tile([P, 512], bf, tag="pt", name="pt011")
        dve_exp(pt011[:], s011[:])

        rounds = [(0, 0), (0, 1), (1, 0), (1, 1)]
        bg = []          # background FIFO: fns emitting pz-chained work
        pending_norm = None

        def transp_subs(p, icb, onst, subs):
            """DMA xbar transposes: onst[sub] [128 i,128 d] -> ot[p] blocks."""
            for sub in subs:
                dst = ot[p][:, icb * ICB + sub * P: icb * ICB + (sub + 1) * P]
                nc.sync.dma_start_transpose(dst, onst[sub][:])

        def do_normalize(ob, p, icb, last):
            """Normalize the finished round's O accumulators -> onst bf16,
            then stage O^T into ot (DMA xbar mid-kernel, PE+DVE at the tail).
            """
            onst = [on_pool.tile([P, P], bf, tag="on", name=f"on{s}")
                    for s in range(NSUB)]
            for h in range(2):
                # batched reciprocals: subs 0-6 rowsums (col 64 of each
                # 65-col slice in bank ob[h]) in one strided op, sub 7 single
                rc8 = rc_pool.tile([P, 8], f32, tag="rc8", name="rc8")
                in7 = ob[h][:, 0:7 * (D + 1)].rearrange(
                    "p (s c) -> p s c", c=D + 1)[:, :, D:D + 1]
                out7 = rc8[:, 0:7].rearrange("p (s c) -> p s c", c=1)
                nc.vector.reciprocal(out7, in7)
                nc.vector.reciprocal(
                    rc8[:, 7:8], ob[2][:, h * (D + 1) + D: h * (D + 1) + D + 1])
                for sub in range(NSUB):
                    sl = ob_slice(ob, h, sub)
                    rc = rc8[:, sub:sub + 1]
                    dst = onst[sub][:, h * D:(h + 1) * D]
                    if h == 0:
                        nc.scalar.activation(dst, sl[:, 0:D], AF.Copy, scale=rc)
                    else:
                        nc.vector.tensor_scalar_mul(dst, sl[:, 0:D], rc)
            if not last:
                # O^T via DMA xbar transpose (latency hidden; DMA is idle)
                transp_subs(p, icb, onst, range(NSUB))
            else:
                # tail: PE is_transpose into one bf16 PSUM bank + two 2x-mode
                # DVE copies (halves, so z ch0 can start after the first)
                tp = pz_pool.tile([P, ICB], bf, tag="pz", name="tptail")
                for sub in range(NSUB):
                    nc.tensor.matmul(
                        tp[:, sub * P:(sub + 1) * P],
                        onst[sub][:],
                        ident[:],
                        is_transpose=True,
                        start=(sub == 0),
                        stop=(sub == NSUB - 1),
                        skip_group_check=True,
                    )
                nc.vector.tensor_copy(
                    ot[p][:, icb * ICB:icb * ICB + 512], tp[:, 0:512])
                nc.vector.tensor_copy(
                    ot[p][:, icb * ICB + 512:(icb + 1) * ICB], tp[:, 512:ICB])

        for r, (icb, p) in enumerate(rounds):
            # load this round's background work (deadline-ordered)
            if r == 0:
                bg += [
                    lambda: proj_v_pair(0, pz_pool),
                    lambda: proj_v_pair(2, pz_pool),
                    lambda: proj_v_pair(4, pz_pool),
                    lambda: proj_v_pair(6, pz_pool),
                    lambda: proj_v_pair(8, pz_pool),
                    lambda: proj_qk_half(0, wk, bk_sb, kt_sb, 1, 0, pz_pool),
                    lambda: proj_qk_half(0, wk, bk_sb, kt_sb, 1, 1, pz_pool),
                    lambda: proj_v_pair(10, pz_pool),
                    lambda: proj_v_pair(12, pz_pool),
                    lambda: proj_qk_half(1, wk, bk_sb, kt_sb, 0, 0, pz_pool,
                                         bias_eng="act"),
                    lambda: proj_qk_half(1, wq, bq_sb, qt, 0, 0, pz_pool,
                                         bias_eng="act"),
                    lambda: proj_v_pair(14, pz_pool),
                    lambda: proj_qk_half(1, wk, bk_sb, kt_sb, 0, 1, pz_pool,
                                         bias_eng="act"),
                    lambda: proj_qk_half(1, wq, bq_sb, qt, 0, 1, pz_pool,
                                         bias_eng="act"),
                ]
            elif r == 1:
                bg += [
                    lambda: proj_qk_half(1, wk, bk_sb, kt_sb, 1, 0, pz_pool,
                                         bias_eng="act"),
                    lambda: proj_qk_half(1, wk, bk_sb, kt_sb, 1, 1, pz_pool,
                                         bias_eng="act"),
                    lambda: proj_qk_half(0, wq, bq_sb, qt, 1, 0, pz_pool,
                                         bias_eng="act"),
                    lambda: proj_qk_half(0, wq, bq_sb, qt, 1, 1, pz_pool,
                                         bias_eng="act"),
                    lambda: proj_qk_half(1, wq, bq_sb, qt, 1, 0, pz_pool,
                                         bias_eng="act"),
                    lambda: proj_qk_half(1, wq, bq_sb, qt, 1, 1, pz_pool,
                                         bias_eng="act"),
                ]
            elif r == 2:
                bg += [None] * 4
                bg += [lambda ft=ft, ch=ch: z_chunk(
                            0, ft, ch, pz_pool,
                            copy_eng=("act" if ft % 2 == 0 else "dve"))
                       for ft in range(F // P) for ch in range(2)]

            if r == 0:
                pts = {(0, 0): pt000, (0, 1): pt001,
                       (1, 0): pt010, (1, 1): pt011}
            else:
                pts = {}
                for h in range(2):
                    for half in range(2):
                        sp = emit_S_half(p, icb, 0, h, half)
                        ptile = pt_pool.tile([P, 512], bf, tag="pt", name="pt")
                        emit_exp(ptile[:], sp[:], exp_eng(r, 0, h))
                        pts[(h, half)] = ptile
            # normalize + transposes of the PREVIOUS round, emitted after
            # jt0's exps so the exp engines prioritize feeding PV(jt0)
            if pending_norm is not None:
                do_normalize(*pending_norm, last=False)
                pending_norm = None
            ob = [po_pool.tile([P, 512], f32, tag="po", name=f"ob{i}")
                  for i in range(3)]
            for jt in range(NT):
                nxt_s = {}
                if jt + 1 < NT:
                    nxt_s[(0, 0)] = emit_S_half(p, icb, jt + 1, 0, 0)
                    nxt_s[(0, 1)] = emit_S_half(p, icb, jt + 1, 0, 1)
                    # slot 1: only pop when backlog exceeds remaining jts
                    if bg and len(bg) > (NT - jt):
                        item = bg.pop(0)
                        if item is not None:
                            item()
                    nxt_s[(1, 0)] = emit_S_half(p, icb, jt + 1, 1, 0)
                    nxt_s[(1, 1)] = emit_S_half(p, icb, jt + 1, 1, 1)
                if bg:
                    item = bg.pop(0)
                    if item is not None:
                        item()
                nxt_pts = {}
                for key, sp in nxt_s.items():
                    ptile = pt_pool.tile([P, 512], bf, tag="pt", name="pt")
                    emit_exp(ptile[:], sp[:], exp_eng(r, jt + 1, key[0]))
                    nxt_pts[key] = ptile
                for h in range(2):
                    hl = 2 * p + h
                    for sub in range(NSUB):
                        ptile = pts[(h, sub // 4)]
                        # start=True zeroes the whole 2KB psum bank row, so
                        # only the first matmul into each ob bank may set it
                        first_in_bank = (sub == 0) or (sub == 7 and h == 0)
                        nc.tensor.matmul(
                            ob_slice(ob, h, sub),
                            ptile[:, (sub % 4) * P:(sub % 4 + 1) * P],
                            v_sb[jt][:, hl * (D + 1):(hl + 1) * (D + 1)],
                            start=(jt == 0 and first_in_bank),
                            stop=(jt == NT - 1),
                            skip_group_check=True,
                        )
                pts = nxt_pts

            pending_norm = (ob, p, icb)

        # ---- final normalize + tail: icb=1 z chunks -------------------------
        do_normalize(*pending_norm, last=True)
        # ch0 chunks first (they only need the first half of r3's O^T copy)
        z_order = [(0, 0), (1, 0), (2, 0), (3, 0), (0, 1), (1, 1), (2, 1), (3, 1)]
        for i, (ft, ch) in enumerate(z_order):
            z_chunk(1, ft, ch, ps_s if i % 5 != 4 else pz_pool,
                    copy_eng=("act" if (ft + ch) % 2 == 0 else "dve"),
                    dma_eng=("act", None, "act", None)[ft])

    nc.compile()
    return nc


def _get_nc():
    if "nc" not in _cache:
        _cache["nc"] = build()
    return _cache["nc"]


def make_in_maps(x, Wq, bq, Wk, bk, Wv, bv, Wo, bo):
    """Host-side sharding: per-core input dict for core c = 2*b + hg."""
    bfnp = _bf_np()
    in_maps = []
    for c in range(8):
        b, hg = divmod(c, 2)
        cs = slice(hg * DH, (hg + 1) * DH)
        wo_s = np.ascontiguousarray(Wo[cs, :])
        in_maps.append({
            "xtb": np.ascontiguousarray(np.asarray(x[b]).T.astype(bfnp)),
            "wkqv": np.ascontiguousarray(
                np.concatenate([Wk[:, cs], Wq[:, cs], Wv[:, cs]], axis=1)
                .astype(bfnp)),
            "wob": np.ascontiguousarray(wo_s.astype(bfnp)),
            "bias4": np.ascontiguousarray(np.concatenate([
                np.asarray(bk[cs]).reshape(NPAIR, P).T,
                np.asarray(bq[cs]).reshape(NPAIR, P).T,
            ], axis=1).astype(np.float32)),
            "ident": np.eye(P, dtype=bfnp),
        })
    return in_maps


def kernel(x, Wq, bq, Wk, bk, Wv, bv, Wo, bo):
    from concourse.bass_utils import run_bass_kernel_spmd

    x = np.asarray(x, dtype=np.float32)
    args = [np.asarray(a, dtype=np.float32) for a in (Wq, bq, Wk, bk, Wv, bv, Wo, bo)]
    Wv_, bv_, Wo_, bo_ = args[4], args[5], args[6], args[7]
    nc = _get_nc()
    in_maps = make_in_maps(x, *args)
    res = run_bass_kernel_spmd(nc, in_maps, list(range(8)))
    zbias = (bv_ @ Wo_ + bo_).astype(np.float32)
    out = np.empty((B, N, F), dtype=np.float32)
    for b in range(B):
        zt0 = np.asarray(res.results[2 * b]["zt"], dtype=np.float32)
        zt1 = np.asarray(res.results[2 * b + 1]["zt"], dtype=np.float32)
        out[b] = (zt0 + zt1).T + zbias
    return out


# revision 20
# speedup vs baseline: 1.0190x; 1.0190x over previous
"""Multi-head self-attention TRN2 Bass kernel.

Problem: x[4,2048,512], 8 heads of d=64, scale 1/sqrt(512) (full feature dim).

Sharding: 8 cores = (batch b in 0..3) x (head-group hg in 0..1). Each core
handles one batch element and 4 heads (256 of the 512 features), computing a
partial output projection z_partial = attn_heads @ Wo[hg rows].  The host
sums the two partials per batch and adds (bv @ Wo + bo).

Engine split (PE is the bottleneck; exp is split across ACT and DVE):
  - S^T tiles [128 j, 1024 i] on PE (f32r, two heads packed via
    tile_position), PV accumulation on PE with a ones-column rowsum.
  - exp: head 0 tiles via ScalarE AF.Exp; head 1 tiles via a custom DVE op
    EXP_POLY16_ANT computing ((x*a2+a1)*x+a0)^16 (deg-2 minimax of
    exp(x*SCALE/16), then 4 squarings; one 8-stage DVE instruction).
  - O_norm [128 i, 128 d] -> O^T transposes via DMA xbar transpose
    (dma_start_transpose), straight into the ot[k] staging layout.
  - z chunks: PE matmul -> ACT copy -> DMA out (biases folded to host).
  - V-ones memsets on GPSIMD (SBUF only); V copies + normalize split DVE/ACT.

Scheduling: per jt, S(jt+1) is emitted before PV(jt); background FIFO items
(projections, z chunks) pop once per jt through the single spare PSUM bank.
"""

import sys
import os

sys.path.insert(0, "/opt/trn_rl_repo")

import numpy as np

B, N, F = 4, 2048, 512
H, D = 8, 64
P = 128
DH = 256   # features per core (4 heads)
NPAIR = 2  # head pairs per core
KT = F // P          # 4 k-tiles over input features
ICB = 1024           # i-chunk per round
NICB = N // ICB      # 2
NT = N // P          # 16 j-tiles
NSUB = ICB // P      # 8 i-subtiles per chunk
SCALE = 1.0 / float(np.float32(F) ** 0.5)

# deg-2 minimax of exp(y) on |y| <= 0.208, y = s*SCALE/16; exp = p^16.
# Fit in fit_poly.py: rel err 5.7e-4 -> 9.2e-3 after ^16 (worst case |S|=75).
EXPC = (1.0000015, 0.0027809313, 3.828529e-06)  # a0, a1*k, a2*k^2 folded

_cache = {}


def _bf_np():
    import ml_dtypes

    return np.dtype(ml_dtypes.bfloat16)


def _exp_op():
    """Register (once) and return the custom DVE poly-exp op."""
    if "exp_op" in _cache:
        return _cache["exp_op"]
    from concourse.dve_spec import Spec, Src0, C0, C1, C2, sq
    from concourse import dve_ops

    def ref(in0, in1, c0, c1, c2):
        x = in0.astype(np.float32)
        p = (x * c2 + c1) * x + c0
        for _ in range(4):
            p = p * p
        return p

    for op in dve_ops.OPS:
        if op.name == "EXP_POLY16_ANT":
            _cache["exp_op"] = op
            return op
    op = dve_ops.DveOp(
        "EXP_POLY16_ANT",
        Spec(body=sq(sq(sq(sq((Src0 * C2 + C1) * Src0 + C0)))), reference=ref),
        subdim=False,
        uops_sha={"v3": "571513505c6c4e44", "v4": "f56cfa9f44bcd3e4"},
    )
    dve_ops.OPS.append(op)
    dve_ops.CUSTOM_DVE_SPECS[op.name] = op.spec
    dve_ops._SUB_OPCODE_FOR_NAME[op.name] = max(
        dve_ops._SUB_OPCODE_FOR_NAME.values()) + 1
    _cache["exp_op"] = op
    return op


def build():
    """Build + bass-compile the per-core program."""
    import concourse.tile as tile
    from concourse import bacc, mybir
    from contextlib import ExitStack

    f32 = mybir.dt.float32
    f32r = mybir.dt.float32r
    bf = mybir.dt.bfloat16
    AF = mybir.ActivationFunctionType
    EXP_OP = _exp_op()

    n = N
    nc = bacc.Bacc("TRN2", target_bir_lowering=False, debug=False)

    xt_d = nc.dram_tensor("xtb", [F, n], bf, kind="ExternalInput").ap()
    wkqv_d = nc.dram_tensor("wkqv", [F, 3 * DH], bf, kind="ExternalInput").ap()
    wo_d = nc.dram_tensor("wob", [DH, F], bf, kind="ExternalInput").ap()
    # packed biases: cols 0-1 bk, 2-3 bq
    bias_d = nc.dram_tensor("bias4", [P, 4], f32, kind="ExternalInput").ap()
    id_d = nc.dram_tensor("ident", [P, P], bf, kind="ExternalInput").ap()
    zt_d = nc.dram_tensor("zt", [F, n], bf, kind="ExternalOutput").ap()

    def dve_exp(dst, src):
        nc.vector._custom_dve(EXP_OP, out=dst, in0=src,
                              s0=EXPC[0], s1=EXPC[1], imm2=EXPC[2])

    with tile.TileContext(nc) as tc, ExitStack() as ctx:
        const = ctx.enter_context(tc.tile_pool(name="const", bufs=1))
        pt_pool = ctx.enter_context(tc.tile_pool(name="pt", bufs=14))
        rc_pool = ctx.enter_context(tc.tile_pool(name="rc", bufs=8))
        on_pool = ctx.enter_context(tc.tile_pool(name="on", bufs=16))
        zs_pool = ctx.enter_context(tc.tile_pool(name="zs", bufs=4))
        ps_s = ctx.enter_context(tc.tile_pool(name="ps_s", bufs=4, space="PSUM"))
        po_pool = ctx.enter_context(tc.tile_pool(name="po", bufs=3, space="PSUM"))
        pz_pool = ctx.enter_context(tc.tile_pool(name="pz", bufs=1, space="PSUM"))

        # ---- DMA loads ------------------------------------------------------
        xt = [const.tile([P, n], bf, tag=f"xt{k}", name=f"xt{k}") for k in range(KT)]
        wkqv = [const.tile([P, 3 * DH], bf, tag=f"wkqv{k}", name=f"wkqv{k}")
                for k in range(KT)]
        wk = [wkqv[k][:, 0:DH] for k in range(KT)]
        wq = [wkqv[k][:, DH:2 * DH] for k in range(KT)]
        wv = [wkqv[k][:, 2 * DH:3 * DH] for k in range(KT)]
        bias_sb = const.tile([P, 4], f32, tag="bias4", name="bias_sb")
        bk_sb = bias_sb[:, 0:NPAIR]
        bq_sb = bias_sb[:, NPAIR:2 * NPAIR]
        # warm tiles first: a dummy matmul starts the PE p-state ramp at t~1us
        # so the prologue projections run at full clock; the exp warms the
        # ScalarE table while DMAs stream in
        warm = const.tile([1, 1], f32, tag="warm", name="warm")
        nc.gpsimd.memset(warm[:], 0.0)
        warmmm = pz_pool.tile([P, 512], f32, tag="pz", name="warmmm")
        nc.tensor.matmul(warmmm[0:1, 0:1], warm[:], warm[:],
                         start=True, stop=True)
        nc.scalar.activation(warm[:], warm[:], AF.Exp)

        # first wave: weights + the first 512 cols of each k-tile (enough for
        # the q/k half-0 projections and j-tiles 0-3) to minimize the
        # DMA-serialized latency to the first S tile
        nc.sync.dma_start(wkqv[0][:], wkqv_d[0:P, :])
        nc.sync.dma_start(xt[0][:, 0:512], xt_d[0:P, 0:512])
        nc.sync.dma_start(bias_sb[:], bias_d[:])
        for k in range(1, KT):
            nc.sync.dma_start(wkqv[k][:], wkqv_d[k * P:(k + 1) * P, :])
            nc.sync.dma_start(xt[k][:, 0:512], xt_d[k * P:(k + 1) * P, 0:512])
        for k in range(KT):
            nc.sync.dma_start(xt[k][:, 512:ICB], xt_d[k * P:(k + 1) * P, 512:ICB])
        for k in range(KT):
            nc.sync.dma_start(xt[k][:, 1024:n], xt_d[k * P:(k + 1) * P, 1024:n])
        wo = [const.tile([P, F], bf, tag=f"wo{k}", name=f"wo{k}") for k in range(DH // P)]
        for k in range(DH // P):
            nc.sync.dma_start(wo[k][:], wo_d[k * P:(k + 1) * P, :])
        ident = const.tile([P, P], bf, tag="ident", name="ident")
        nc.sync.dma_start(ident[:], id_d[:])

        # persistent activations
        qt = [const.tile([P, n], f32r, tag=f"qt{p}", name=f"qt{p}") for p in range(NPAIR)]
        kt_sb = [const.tile([P, n], f32r, tag=f"kt{p}", name=f"ktsb{p}") for p in range(NPAIR)]
        # V per j-tile: [128, 260] bf16, head hl at cols [65*hl, 65*hl+64),
        # ones at col 65*hl+64 (accumulates softmax row-sums in PV).
        v_sb = [const.tile([P, 4 * (D + 1)], bf, tag=f"v{j}", name=f"v{j}")
                for j in range(NT)]
        for j in range(NT):
            nc.gpsimd.memset(v_sb[j][:], 1.0)
        # O^T staging for the z projection: [128 d, n] bf16 per k-tile (pair)
        ot = [const.tile([P, n], bf, tag=f"ot{p}", name=f"ot{p}") for p in range(NPAIR)]

        def v4(ap):
            return ap.rearrange("p (h c) -> p h c", h=4)

        # ---- projections -----------------------------------------------------
        def proj_qk_half(p, w_t, b_sb, dst, ib, half, pool, bias_eng="vector"):
            """One 512-wide half of a Q/K projection chunk."""
            ps = pool.tile([P, 512], f32, tag="pz", name="pjh") if pool is pz_pool \
                else pool.tile([P, 512], f32, tag="st", name="pjs")
            psl = ps[:, 0:512]
            isl = slice(ib * ICB + half * 512, ib * ICB + (half + 1) * 512)
            for k in range(KT):
                nc.tensor.matmul(
                    psl,
                    w_t[k][:, p * P:(p + 1) * P],
                    xt[k][:, isl],
                    start=(k == 0),
                    stop=(k == KT - 1),
                )
            if bias_eng == "act":
                nc.scalar.activation(dst[p][:, isl], psl, AF.Identity,
                                     bias=b_sb[:, p:p + 1])
            else:
                nc.vector.tensor_scalar_add(dst[p][:, isl], psl, b_sb[:, p:p + 1])

        def proj_v_pair(j0, pool):
            """V for j-tiles j0, j0+1 sharing one pz bank (two 256-col slices)."""
            ps = pool.tile([P, 512], f32, tag="pz", name="pvh") if pool is pz_pool \
                else pool.tile([P, 512], f32, tag="st", name="pvs")
            for m, j in enumerate((j0, j0 + 1)):
                psl = ps[:, m * DH:(m + 1) * DH]
                for k in range(KT):
                    nc.tensor.matmul(
                        psl,
                        xt[k][:, j * P:(j + 1) * P],
                        wv[k][:],
                        start=(k == 0 and m == 0),
                        stop=(k == KT - 1 and m == 1),
                        skip_group_check=True,
                    )
            for m, j in enumerate((j0, j0 + 1)):
                psl = ps[:, m * DH:(m + 1) * DH]
                nc.vector.tensor_copy(v4(v_sb[j][:])[:, :, 0:D], v4(psl))

        zsb_by_ft = {}

        def z_chunk(icb, ft, ch, pool, copy_eng="act", dma_eng=None):
            """z^T[ft*128:(ft+1)*128, 512-chunk ch of icb]; both chunks of an
            (icb, ft) share one [128,1024] bf16 staging tile and one DMA."""
            tag = {id(pz_pool): "pz", id(po_pool): "po"}.get(id(pool), "st")
            zp = pool.tile([P, 512], f32, tag=tag, name="zp")
            zpl = zp[:, 0:512]
            isl = slice(icb * ICB + ch * 512, icb * ICB + (ch + 1) * 512)
            for k in range(DH // P):
                nc.tensor.matmul(
                    zpl,
                    wo[k][:, ft * P:(ft + 1) * P],
                    ot[k][:, isl],
                    start=(k == 0),
                    stop=(k == DH // P - 1),
                )
            if (icb, ft) not in zsb_by_ft:
                zsb_by_ft[(icb, ft)] = zs_pool.tile([P, ICB], bf, tag="zt",
                                                    name="zsb")
            zsb = zsb_by_ft[(icb, ft)]
            if copy_eng == "act":
                nc.scalar.activation(zsb[:, ch * 512:(ch + 1) * 512], zpl, AF.Copy)
            else:
                nc.vector.tensor_copy(zsb[:, ch * 512:(ch + 1) * 512], zpl)
            if ch == 1:
                eng = {None: nc.sync, "act": nc.scalar}[dma_eng]
                eng.dma_start(
                    zt_d[ft * P:(ft + 1) * P, icb * ICB:(icb + 1) * ICB], zsb[:])

        # ---- attention rounds ------------------------------------------------
        def ob_slice(ob, h, sub):
            """PSUM accumulator slice [128, 65] for (head h, i-subtile sub)."""
            if sub < 7:
                t = ob[h]
                c0 = sub * (D + 1)
            else:
                t = ob[2]
                c0 = h * (D + 1)
            return t[:, c0:c0 + D + 1]

        def emit_S_half(p, icb, jt, h, half):
            sps = ps_s.tile([P, 512], f32, tag="st", name="sps")
            hp = slice(D * h, D * (h + 1))
            isl = slice(icb * ICB + half * 512, icb * ICB + (half + 1) * 512)
            nc.tensor.matmul(
                sps[:, 0:512],
                kt_sb[p][hp, jt * P:(jt + 1) * P],
                qt[p][hp, isl],
                start=True,
                stop=True,
                tile_position=(D * h, 0),
            )
            return sps

        def emit_exp(dst, src, eng):
            if eng == "act":
                nc.scalar.activation(dst, src, AF.Exp, scale=SCALE)
            else:
                dve_exp(dst, src)

        # exp routing: head 0 -> ACT, head 1 -> DVE (tunable per round)
        def exp_eng(r, jt, h):
            return "act" if h == 0 else "dve"

        # prologue: K/Q first chunks + jt0's four S halves + the FIRST EXPS.
        proj_qk_half(0, wq, bq_sb, qt, 0, 0, ps_s, bias_eng="act")
        proj_qk_half(0, wk, bk_sb, kt_sb, 0, 0, ps_s)
        s000 = emit_S_half(0, 0, 0, 0, 0)
        pt000 = pt_pool.tile([P, 512], bf, tag="pt", name="pt000")
        nc.scalar.activation(pt000[:], s000[:], AF.Exp, scale=SCALE)
        proj_qk_half(0, wq, bq_sb, qt, 0, 1, ps_s, bias_eng="act")
        s001 = emit_S_half(0, 0, 0, 0, 1)
        proj_qk_half(0, wk, bk_sb, kt_sb, 0, 1, pz_pool)
        pt001 = pt_pool.tile([P, 512], bf, tag="pt", name="pt001")
        nc.scalar.activation(pt001[:], s001[:], AF.Exp, scale=SCALE)
        s010 = emit_S_half(0, 0, 0, 1, 0)
        s011 = emit_S_half(0, 0, 0, 1, 1)
        pt010 = pt_pool.tile([P, 512], bf, tag="pt", name="pt010")
        dve_exp(pt010[:], s010[:])
        pt011 = pt_pool.tile([P, 512], bf, tag="pt", name="pt011")
        dve_exp(pt011[:], s011[:])

        rounds = [(0, 0), (0, 1), (1, 0), (1, 1)]
        bg = []          # background FIFO: fns emitting pz-chained work
        pending_norm = None

        def transp_subs(p, icb, onst, subs):
            """DMA xbar transposes: onst[sub] [128 i,128 d] -> ot[p] blocks."""
            for sub in subs:
                dst = ot[p][:, icb * ICB + sub * P: icb * ICB + (sub + 1) * P]
                nc.sync.dma_start_transpose(dst, onst[sub][:])

        def do_normalize(ob, p, icb, last):
            """Normalize the finished round's O accumulators -> onst bf16,
            then stage O^T into ot (DMA xbar mid-kernel, PE+DVE at the tail).
            """
            onst = [on_pool.tile([P, P], bf, tag="on", name=f"on{s}")
                    for s in range(NSUB)]
            for h in range(2):
                # batched reciprocals: subs 0-6 rowsums (col 64 of each
                # 65-col slice in bank ob[h]) in one strided op, sub 7 single
                rc8 = rc_pool.tile([P, 8], f32, tag="rc8", name="rc8")
                in7 = ob[h][:, 0:7 * (D + 1)].rearrange(
                    "p (s c) -> p s c", c=D + 1)[:, :, D:D + 1]
                out7 = rc8[:, 0:7].rearrange("p (s c) -> p s c", c=1)
                nc.vector.reciprocal(out7, in7)
                nc.vector.reciprocal(
                    rc8[:, 7:8], ob[2][:, h * (D + 1) + D: h * (D + 1) + D + 1])
                for sub in range(NSUB):
                    sl = ob_slice(ob, h, sub)
                    rc = rc8[:, sub:sub + 1]
                    dst = onst[sub][:, h * D:(h + 1) * D]
                    if h == 0:
                        nc.scalar.activation(dst, sl[:, 0:D], AF.Copy, scale=rc)
                    else:
                        nc.vector.tensor_scalar_mul(dst, sl[:, 0:D], rc)
            if not last:
                # O^T via DMA xbar transpose (latency hidden; DMA is idle)
                transp_subs(p, icb, onst, range(NSUB))
            else:
                # tail: PE is_transpose into one bf16 PSUM bank + two 2x-mode
                # DVE copies (halves, so z ch0 can start after the first)
                tp = pz_pool.tile([P, ICB], bf, tag="pz", name="tptail")
                for sub in range(NSUB):
                    nc.tensor.matmul(
                        tp[:, sub * P:(sub + 1) * P],
                        onst[sub][:],
                        ident[:],
                        is_transpose=True,
                        start=(sub == 0),
                        stop=(sub == NSUB - 1),
                        skip_group_check=True,
                    )
                nc.vector.tensor_copy(
                    ot[p][:, icb * ICB:icb * ICB + 512], tp[:, 0:512])
                nc.vector.tensor_copy(
                    ot[p][:, icb * ICB + 512:(icb + 1) * ICB], tp[:, 512:ICB])

        for r, (icb, p) in enumerate(rounds):
            # load this round's background work (deadline-ordered)
            if r == 0:
                bg += [
                    lambda: proj_v_pair(0, pz_pool),
                    lambda: proj_v_pair(2, pz_pool),
                    lambda: proj_v_pair(4, pz_pool),
                    lambda: proj_v_pair(6, pz_pool),
                    lambda: proj_v_pair(8, pz_pool),
                    lambda: proj_qk_half(0, wk, bk_sb, kt_sb, 1, 0, pz_pool),
                    lambda: proj_qk_half(0, wk, bk_sb, kt_sb, 1, 1, pz_pool),
                    lambda: proj_v_pair(10, pz_pool),
                    lambda: proj_v_pair(12, pz_pool),
                    lambda: proj_qk_half(1, wk, bk_sb, kt_sb, 0, 0, pz_pool,
                                         bias_eng="act"),
                    lambda: proj_qk_half(1, wq, bq_sb, qt, 0, 0, pz_pool,
                                         bias_eng="act"),
                    lambda: proj_v_pair(14, pz_pool),
                    lambda: proj_qk_half(1, wk, bk_sb, kt_sb, 0, 1, pz_pool,
                                         bias_eng="act"),
                    lambda: proj_qk_half(1, wq, bq_sb, qt, 0, 1, pz_pool,
                                         bias_eng="act"),
                ]
            elif r == 1:
                bg += [
                    lambda: proj_qk_half(1, wk, bk_sb, kt_sb, 1, 0, pz_pool,
                                         bias_eng="act"),
                    lambda: proj_qk_half(1, wk, bk_sb, kt_sb, 1, 1, pz_pool,
                                         bias_eng="act"),
                    lambda: proj_qk_half(0, wq, bq_sb, qt, 1, 0, pz_pool,
                                         bias_eng="act"),
                    lambda: proj_qk_half(0, wq, bq_sb, qt, 1, 1, pz_pool,
                                         bias_eng="act"),
                    lambda: proj_qk_half(1, wq, bq_sb, qt, 1, 0, pz_pool,
                                         bias_eng="act"),
                    lambda: proj_qk_half(1, wq, bq_sb, qt, 1, 1, pz_pool,
                                         bias_eng="act"),
                ]
            elif r == 2:
                bg += [None] * 4
                bg += [lambda ft=ft, ch=ch: z_chunk(
                            0, ft, ch, pz_pool,
                            copy_eng=("act" if ft % 2 == 0 else "dve"))
                       for ft in range(F // P) for ch in range(2)]

            if r == 0:
                pts = {(0, 0): pt000, (0, 1): pt001,
                       (1, 0): pt010, (1, 1): pt011}
            else:
                pts = {}
                for h in range(2):
                    for half in range(2):
                        sp = emit_S_half(p, icb, 0, h, half)
                        ptile = pt_pool.tile([P, 512], bf, tag="pt", name="pt")
                        emit_exp(ptile[:], sp[:], exp_eng(r, 0, h))
                        pts[(h, half)] = ptile
            # normalize + transposes of the PREVIOUS round, emitted after
            # jt0's exps so the exp engines prioritize feeding PV(jt0)
            if pending_norm is not None:
                do_normalize(*pending_norm, last=False)
                pending_norm = None
            ob = [po_pool.tile([P, 512], f32, tag="po", name=f"ob{i}")
                  for i in range(3)]
            def emit_PV(jt, pts_jt):
                for h in range(2):
                    hl = 2 * p + h
                    for sub in range(NSUB):
                        ptile = pts_jt[(h, sub // 4)]
                        # start=True zeroes the whole 2KB psum bank row, so
                        # only the first matmul into each ob bank may set it
                        first_in_bank = (sub == 0) or (sub == 7 and h == 0)
                        nc.tensor.matmul(
                            ob_slice(ob, h, sub),
                            ptile[:, (sub % 4) * P:(sub % 4 + 1) * P],
                            v_sb[jt][:, hl * (D + 1):(hl + 1) * (D + 1)],
                            start=(jt == 0 and first_in_bank),
                            stop=(jt == NT - 1),
                            skip_group_check=True,
                        )

            # PV lags one jt behind S/exp emission so exp-engine backlogs
            # (normalize bursts at round boundaries) never stall PE
            pts_prev = None
            for jt in range(NT):
                nxt_pts = None
                if jt + 1 < NT:
                    nxt_s = {}
                    nxt_s[(0, 0)] = emit_S_half(p, icb, jt + 1, 0, 0)
                    nxt_s[(0, 1)] = emit_S_half(p, icb, jt + 1, 0, 1)
                    # slot 1: only pop when backlog exceeds remaining jts
                    if bg and len(bg) > (NT - jt):
                        item = bg.pop(0)
                        if item is not None:
                            item()
                    nxt_s[(1, 0)] = emit_S_half(p, icb, jt + 1, 1, 0)
                    nxt_s[(1, 1)] = emit_S_half(p, icb, jt + 1, 1, 1)
                    nxt_pts = {}
                    for key, sp in nxt_s.items():
                        ptile = pt_pool.tile([P, 512], bf, tag="pt", name="pt")
                        emit_exp(ptile[:], sp[:], exp_eng(r, jt + 1, key[0]))
                        nxt_pts[key] = ptile
                if bg:
                    item = bg.pop(0)
                    if item is not None:
                        item()
                if jt >= 1:
                    emit_PV(jt - 1, pts_prev)
                pts_prev, pts = pts, nxt_pts
            emit_PV(NT - 1, pts_prev)

            pending_norm = (ob, p, icb)

        # ---- final normalize + tail: icb=1 z chunks -------------------------
        do_normalize(*pending_norm, last=True)
        # ch0 chunks first (they only need the first half of r3's O^T copy);
        # all 8 psum banks (ps_s 4 + po 3 + pz 1) so no slot-recycle waits
        z_order = [(0, 0), (1, 0), (2, 0), (3, 0), (0, 1), (1, 1), (2, 1), (3, 1)]
        z_pools = [ps_s, ps_s, ps_s, ps_s, po_pool, po_pool, po_pool, pz_pool]
        for i, (ft, ch) in enumerate(z_order):
            z_chunk(1, ft, ch, z_pools[i],
                    copy_eng=("act" if (ft + ch) % 2 == 0 else "dve"),
                    dma_eng=("act", None, "act", None)[ft])

    nc.compile()
    return nc


def _get_nc():
    if "nc" not in _cache:
        _cache["nc"] = build()
    return _cache["nc"]


def make_in_maps(x, Wq, bq, Wk, bk, Wv, bv, Wo, bo):
    """Host-side sharding: per-core input dict for core c = 2*b + hg."""
    bfnp = _bf_np()
    in_maps = []
    for c in range(8):
        b, hg = divmod(c, 2)
        cs = slice(hg * DH, (hg + 1) * DH)
        wo_s = np.ascontiguousarray(Wo[cs, :])
        in_maps.append({
            "xtb": np.ascontiguousarray(np.asarray(x[b]).T.astype(bfnp)),
            "wkqv": np.ascontiguousarray(
                np.concatenate([Wk[:, cs], Wq[:, cs], Wv[:, cs]], axis=1)
                .astype(bfnp)),
            "wob": np.ascontiguousarray(wo_s.astype(bfnp)),
            "bias4": np.ascontiguousarray(np.concatenate([
                np.asarray(bk[cs]).reshape(NPAIR, P).T,
                np.asarray(bq[cs]).reshape(NPAIR, P).T,
            ], axis=1).astype(np.float32)),
            "ident": np.eye(P, dtype=bfnp),
        })
    return in_maps


def kernel(x, Wq, bq, Wk, bk, Wv, bv, Wo, bo):
    from concourse.bass_utils import run_bass_kernel_spmd

    x = np.asarray(x, dtype=np.float32)
    args = [np.asarray(a, dtype=np.float32) for a in (Wq, bq, Wk, bk, Wv, bv, Wo, bo)]
    Wv_, bv_, Wo_, bo_ = args[4], args[5], args[6], args[7]
    nc = _get_nc()
    in_maps = make_in_maps(x, *args)
    res = run_bass_kernel_spmd(nc, in_maps, list(range(8)))
    zbias = (bv_ @ Wo_ + bo_).astype(np.float32)
    out = np.empty((B, N, F), dtype=np.float32)
    for b in range(B):
        zt0 = np.asarray(res.results[2 * b]["zt"], dtype=np.float32)
        zt1 = np.asarray(res.results[2 * b + 1]["zt"], dtype=np.float32)
        out[b] = (zt0 + zt1).T + zbias
    return out


# revision 21
# speedup vs baseline: 1.0247x; 1.0056x over previous
"""Multi-head self-attention TRN2 Bass kernel.

Problem: x[4,2048,512], 8 heads of d=64, scale 1/sqrt(512) (full feature dim).

Sharding: 8 cores = (batch b in 0..3) x (head-group hg in 0..1). Each core
handles one batch element and 4 heads (256 of the 512 features), computing a
partial output projection z_partial = attn_heads @ Wo[hg rows].  The host
sums the two partials per batch and adds (bv @ Wo + bo).

Engine split (PE is the bottleneck; exp is split across ACT and DVE):
  - S^T tiles [128 j, 1024 i] on PE (f32r, two heads packed via
    tile_position), PV accumulation on PE with a ones-column rowsum.
  - exp: head 0 tiles via ScalarE AF.Exp; head 1 tiles via a custom DVE op
    EXP_POLY16_ANT computing ((x*a2+a1)*x+a0)^16 (deg-2 minimax of
    exp(x*SCALE/16), then 4 squarings; one 8-stage DVE instruction).
  - O_norm [128 i, 128 d] -> O^T transposes via DMA xbar transpose
    (dma_start_transpose), straight into the ot[k] staging layout.
  - z chunks: PE matmul -> ACT copy -> DMA out (biases folded to host).
  - V-ones memsets on GPSIMD (SBUF only); V copies + normalize split DVE/ACT.

Scheduling: per jt, S(jt+1) is emitted before PV(jt); background FIFO items
(projections, z chunks) pop once per jt through the single spare PSUM bank.
"""

import sys
import os

sys.path.insert(0, "/opt/trn_rl_repo")

import numpy as np

B, N, F = 4, 2048, 512
H, D = 8, 64
P = 128
DH = 256   # features per core (4 heads)
NPAIR = 2  # head pairs per core
KT = F // P          # 4 k-tiles over input features
ICB = 1024           # i-chunk per round
NICB = N // ICB      # 2
NT = N // P          # 16 j-tiles
NSUB = ICB // P      # 8 i-subtiles per chunk
SCALE = 1.0 / float(np.float32(F) ** 0.5)

# deg-2 minimax of exp(y) on |y| <= 0.208, y = s*SCALE/16; exp = p^16.
# Fit in fit_poly.py: rel err 5.7e-4 -> 9.2e-3 after ^16 (worst case |S|=75).
EXPC = (1.0000015, 0.0027809313, 3.828529e-06)  # a0, a1*k, a2*k^2 folded

_cache = {}


def _bf_np():
    import ml_dtypes

    return np.dtype(ml_dtypes.bfloat16)


def _exp_op():
    """Register (once) and return the custom DVE poly-exp op."""
    if "exp_op" in _cache:
        return _cache["exp_op"]
    from concourse.dve_spec import Spec, Src0, C0, C1, C2, sq
    from concourse import dve_ops

    def ref(in0, in1, c0, c1, c2):
        x = in0.astype(np.float32)
        p = (x * c2 + c1) * x + c0
        for _ in range(4):
            p = p * p
        return p

    for op in dve_ops.OPS:
        if op.name == "EXP_POLY16_ANT":
            _cache["exp_op"] = op
            return op
    op = dve_ops.DveOp(
        "EXP_POLY16_ANT",
        Spec(body=sq(sq(sq(sq((Src0 * C2 + C1) * Src0 + C0)))), reference=ref),
        subdim=False,
        uops_sha={"v3": "571513505c6c4e44", "v4": "f56cfa9f44bcd3e4"},
    )
    dve_ops.OPS.append(op)
    dve_ops.CUSTOM_DVE_SPECS[op.name] = op.spec
    dve_ops._SUB_OPCODE_FOR_NAME[op.name] = max(
        dve_ops._SUB_OPCODE_FOR_NAME.values()) + 1
    _cache["exp_op"] = op
    return op


def build():
    """Build + bass-compile the per-core program."""
    import concourse.tile as tile
    from concourse import bacc, mybir
    from contextlib import ExitStack

    f32 = mybir.dt.float32
    f32r = mybir.dt.float32r
    bf = mybir.dt.bfloat16
    AF = mybir.ActivationFunctionType
    EXP_OP = _exp_op()

    n = N
    nc = bacc.Bacc("TRN2", target_bir_lowering=False, debug=False)

    xt_d = nc.dram_tensor("xtb", [F, n], bf, kind="ExternalInput").ap()
    wkqv_d = nc.dram_tensor("wkqv", [F, 3 * DH], bf, kind="ExternalInput").ap()
    wo_d = nc.dram_tensor("wob", [DH, F], bf, kind="ExternalInput").ap()
    # packed biases: cols 0-1 bk, 2-3 bq
    bias_d = nc.dram_tensor("bias4", [P, 4], f32, kind="ExternalInput").ap()
    id_d = nc.dram_tensor("ident", [P, P], bf, kind="ExternalInput").ap()
    zt_d = nc.dram_tensor("zt", [F, n], bf, kind="ExternalOutput").ap()

    def dve_exp(dst, src):
        nc.vector._custom_dve(EXP_OP, out=dst, in0=src,
                              s0=EXPC[0], s1=EXPC[1], imm2=EXPC[2])

    with tile.TileContext(nc) as tc, ExitStack() as ctx:
        const = ctx.enter_context(tc.tile_pool(name="const", bufs=1))
        pt_pool = ctx.enter_context(tc.tile_pool(name="pt", bufs=14))
        rc_pool = ctx.enter_context(tc.tile_pool(name="rc", bufs=8))
        on_pool = ctx.enter_context(tc.tile_pool(name="on", bufs=16))
        zs_pool = ctx.enter_context(tc.tile_pool(name="zs", bufs=4))
        ps_s = ctx.enter_context(tc.tile_pool(name="ps_s", bufs=4, space="PSUM"))
        po_pool = ctx.enter_context(tc.tile_pool(name="po", bufs=3, space="PSUM"))
        pz_pool = ctx.enter_context(tc.tile_pool(name="pz", bufs=1, space="PSUM"))

        # ---- DMA loads ------------------------------------------------------
        xt = [const.tile([P, n], bf, tag=f"xt{k}", name=f"xt{k}") for k in range(KT)]
        wkqv = [const.tile([P, 3 * DH], bf, tag=f"wkqv{k}", name=f"wkqv{k}")
                for k in range(KT)]
        wk = [wkqv[k][:, 0:DH] for k in range(KT)]
        wq = [wkqv[k][:, DH:2 * DH] for k in range(KT)]
        wv = [wkqv[k][:, 2 * DH:3 * DH] for k in range(KT)]
        bias_sb = const.tile([P, 4], f32, tag="bias4", name="bias_sb")
        bk_sb = bias_sb[:, 0:NPAIR]
        bq_sb = bias_sb[:, NPAIR:2 * NPAIR]
        # warm tiles first: a dummy matmul starts the PE p-state ramp at t~1us
        # so the prologue projections run at full clock; the exp warms the
        # ScalarE table while DMAs stream in
        warm = const.tile([1, 1], f32, tag="warm", name="warm")
        nc.gpsimd.memset(warm[:], 0.0)
        warmmm = pz_pool.tile([P, 512], f32, tag="pz", name="warmmm")
        nc.tensor.matmul(warmmm[0:1, 0:1], warm[:], warm[:],
                         start=True, stop=True)
        nc.scalar.activation(warm[:], warm[:], AF.Exp)

        # first wave: weights + the first 512 cols of each k-tile (enough for
        # the q/k half-0 projections and j-tiles 0-3) to minimize the
        # DMA-serialized latency to the first S tile
        nc.sync.dma_start(wkqv[0][:], wkqv_d[0:P, :])
        nc.sync.dma_start(xt[0][:, 0:512], xt_d[0:P, 0:512])
        nc.sync.dma_start(bias_sb[:], bias_d[:])
        for k in range(1, KT):
            nc.sync.dma_start(wkqv[k][:], wkqv_d[k * P:(k + 1) * P, :])
            nc.sync.dma_start(xt[k][:, 0:512], xt_d[k * P:(k + 1) * P, 0:512])
        for k in range(KT):
            nc.sync.dma_start(xt[k][:, 512:ICB], xt_d[k * P:(k + 1) * P, 512:ICB])
        for k in range(KT):
            nc.sync.dma_start(xt[k][:, 1024:n], xt_d[k * P:(k + 1) * P, 1024:n])
        wo = [const.tile([P, F], bf, tag=f"wo{k}", name=f"wo{k}") for k in range(DH // P)]
        for k in range(DH // P):
            nc.sync.dma_start(wo[k][:], wo_d[k * P:(k + 1) * P, :])
        ident = const.tile([P, P], bf, tag="ident", name="ident")
        nc.sync.dma_start(ident[:], id_d[:])

        # persistent activations
        qt = [const.tile([P, n], f32r, tag=f"qt{p}", name=f"qt{p}") for p in range(NPAIR)]
        kt_sb = [const.tile([P, n], f32r, tag=f"kt{p}", name=f"ktsb{p}") for p in range(NPAIR)]
        # V per j-tile: [128, 260] bf16, head hl at cols [65*hl, 65*hl+64),
        # ones at col 65*hl+64 (accumulates softmax row-sums in PV).
        v_sb = [const.tile([P, 4 * (D + 1)], bf, tag=f"v{j}", name=f"v{j}")
                for j in range(NT)]
        for j in range(NT):
            nc.gpsimd.memset(v_sb[j][:], 1.0)
        # O^T staging for the z projection: [128 d, n] bf16 per k-tile (pair)
        ot = [const.tile([P, n], bf, tag=f"ot{p}", name=f"ot{p}") for p in range(NPAIR)]

        def v4(ap):
            return ap.rearrange("p (h c) -> p h c", h=4)

        # ---- projections -----------------------------------------------------
        def proj_qk_half(p, w_t, b_sb, dst, ib, half, pool, bias_eng="vector"):
            """One 512-wide half of a Q/K projection chunk."""
            ps = pool.tile([P, 512], f32, tag="pz", name="pjh") if pool is pz_pool \
                else pool.tile([P, 512], f32, tag="st", name="pjs")
            psl = ps[:, 0:512]
            isl = slice(ib * ICB + half * 512, ib * ICB + (half + 1) * 512)
            for k in range(KT):
                nc.tensor.matmul(
                    psl,
                    w_t[k][:, p * P:(p + 1) * P],
                    xt[k][:, isl],
                    start=(k == 0),
                    stop=(k == KT - 1),
                )
            if bias_eng == "act":
                nc.scalar.activation(dst[p][:, isl], psl, AF.Identity,
                                     bias=b_sb[:, p:p + 1])
            else:
                nc.vector.tensor_scalar_add(dst[p][:, isl], psl, b_sb[:, p:p + 1])

        def proj_v_pair(j0, pool):
            """V for j-tiles j0, j0+1 sharing one pz bank (two 256-col slices)."""
            ps = pool.tile([P, 512], f32, tag="pz", name="pvh") if pool is pz_pool \
                else pool.tile([P, 512], f32, tag="st", name="pvs")
            for m, j in enumerate((j0, j0 + 1)):
                psl = ps[:, m * DH:(m + 1) * DH]
                for k in range(KT):
                    nc.tensor.matmul(
                        psl,
                        xt[k][:, j * P:(j + 1) * P],
                        wv[k][:],
                        start=(k == 0 and m == 0),
                        stop=(k == KT - 1 and m == 1),
                        skip_group_check=True,
                    )
            for m, j in enumerate((j0, j0 + 1)):
                psl = ps[:, m * DH:(m + 1) * DH]
                nc.vector.tensor_copy(v4(v_sb[j][:])[:, :, 0:D], v4(psl))

        zsb_by_ft = {}

        def z_chunk(icb, ft, ch, pool, copy_eng="act", dma_eng=None):
            """z^T[ft*128:(ft+1)*128, 512-chunk ch of icb]; both chunks of an
            (icb, ft) share one [128,1024] bf16 staging tile and one DMA."""
            tag = {id(pz_pool): "pz", id(po_pool): "po"}.get(id(pool), "st")
            zp = pool.tile([P, 512], f32, tag=tag, name="zp")
            zpl = zp[:, 0:512]
            isl = slice(icb * ICB + ch * 512, icb * ICB + (ch + 1) * 512)
            for k in range(DH // P):
                nc.tensor.matmul(
                    zpl,
                    wo[k][:, ft * P:(ft + 1) * P],
                    ot[k][:, isl],
                    start=(k == 0),
                    stop=(k == DH // P - 1),
                )
            zsb = zs_pool.tile([P, 512], bf, tag="zt", name="zsb")
            if copy_eng == "act":
                nc.scalar.activation(zsb[:], zpl, AF.Copy)
            else:
                nc.vector.tensor_copy(zsb[:], zpl)
            eng = {None: nc.sync, "act": nc.scalar}[dma_eng]
            eng.dma_start(zt_d[ft * P:(ft + 1) * P, isl], zsb[:])

        # ---- attention rounds ------------------------------------------------
        def ob_slice(ob, h, sub):
            """PSUM accumulator slice [128, 65] for (head h, i-subtile sub)."""
            if sub < 7:
                t = ob[h]
                c0 = sub * (D + 1)
            else:
                t = ob[2]
                c0 = h * (D + 1)
            return t[:, c0:c0 + D + 1]

        def emit_S_half(p, icb, jt, h, half):
            sps = ps_s.tile([P, 512], f32, tag="st", name="sps")
            hp = slice(D * h, D * (h + 1))
            isl = slice(icb * ICB + half * 512, icb * ICB + (half + 1) * 512)
            nc.tensor.matmul(
                sps[:, 0:512],
                kt_sb[p][hp, jt * P:(jt + 1) * P],
                qt[p][hp, isl],
                start=True,
                stop=True,
                tile_position=(D * h, 0),
            )
            return sps

        def emit_exp(dst, src, eng):
            if eng == "act":
                nc.scalar.activation(dst, src, AF.Exp, scale=SCALE)
            else:
                dve_exp(dst, src)

        # exp routing: head 0 -> ACT, head 1 -> DVE; jt0 fully on ACT so the
        # boundary normalize (DVE) never delays PV(jt0)
        def exp_eng(r, jt, h):
            return "act" if (h == 0 or jt == 0) else "dve"

        # prologue: K/Q first chunks + jt0's four S halves + the FIRST EXPS.
        proj_qk_half(0, wq, bq_sb, qt, 0, 0, ps_s, bias_eng="act")
        proj_qk_half(0, wk, bk_sb, kt_sb, 0, 0, ps_s)
        s000 = emit_S_half(0, 0, 0, 0, 0)
        pt000 = pt_pool.tile([P, 512], bf, tag="pt", name="pt000")
        nc.scalar.activation(pt000[:], s000[:], AF.Exp, scale=SCALE)
        proj_qk_half(0, wq, bq_sb, qt, 0, 1, ps_s, bias_eng="act")
        s001 = emit_S_half(0, 0, 0, 0, 1)
        proj_qk_half(0, wk, bk_sb, kt_sb, 0, 1, pz_pool)
        pt001 = pt_pool.tile([P, 512], bf, tag="pt", name="pt001")
        nc.scalar.activation(pt001[:], s001[:], AF.Exp, scale=SCALE)
        s010 = emit_S_half(0, 0, 0, 1, 0)
        s011 = emit_S_half(0, 0, 0, 1, 1)
        pt010 = pt_pool.tile([P, 512], bf, tag="pt", name="pt010")
        dve_exp(pt010[:], s010[:])
        pt011 = pt_pool.tile([P, 512], bf, tag="pt", name="pt011")
        dve_exp(pt011[:], s011[:])

        rounds = [(0, 0), (0, 1), (1, 0), (1, 1)]
        bg = []          # background FIFO: fns emitting pz-chained work
        pending_norm = None

        def transp_subs(p, icb, onst, subs):
            """DMA xbar transposes: onst[sub] [128 i,128 d] -> ot[p] blocks."""
            for sub in subs:
                dst = ot[p][:, icb * ICB + sub * P: icb * ICB + (sub + 1) * P]
                nc.sync.dma_start_transpose(dst, onst[sub][:])

        def do_normalize(ob, p, icb, last):
            """Normalize the finished round's O accumulators -> onst bf16,
            then stage O^T into ot (DMA xbar mid-kernel, PE+DVE at the tail).
            Muls are batched (stride-0 broadcast of the rowsum reciprocals)
            so only ~6 DVE ops land at the round boundary."""
            onst_t = on_pool.tile([P, NSUB * P], bf, tag="on", name="onst")
            onst = [onst_t[:, s * P:(s + 1) * P] for s in range(NSUB)]
            for h in range(2):
                # batched reciprocals: subs 0-6 rowsums (col 64 of each
                # 65-col slice in bank ob[h]) in one strided op, sub 7 single
                rc8 = rc_pool.tile([P, 8], f32, tag="rc8", name="rc8")
                in7 = ob[h][:, 0:7 * (D + 1)].rearrange(
                    "p (s c) -> p s c", c=D + 1)[:, :, D:D + 1]
                out7 = rc8[:, 0:7].rearrange("p (s c) -> p s c", c=1)
                nc.vector.reciprocal(out7, in7)
                nc.vector.reciprocal(
                    rc8[:, 7:8], ob[2][:, h * (D + 1) + D: h * (D + 1) + D + 1])
                src7 = ob[h][:, 0:7 * (D + 1)].rearrange(
                    "p (s c) -> p s c", c=D + 1)[:, :, 0:D]
                dst7 = onst_t[:].rearrange(
                    "p (s hh c) -> p s hh c", hh=2, c=D)[:, 0:7, h, :]
                rb7 = rc8[:, 0:7].rearrange(
                    "p (s c) -> p s c", c=1).broadcast_to([P, 7, D])
                nc.vector.tensor_tensor(dst7, src7, rb7, mybir.AluOpType.mult)
                nc.vector.tensor_scalar_mul(
                    onst_t[:, 7 * P + h * D:7 * P + (h + 1) * D],
                    ob[2][:, h * (D + 1):h * (D + 1) + D], rc8[:, 7:8])
            if not last:
                # O^T via DMA xbar transpose (latency hidden; DMA is idle)
                transp_subs(p, icb, onst, range(NSUB))
            else:
                # tail: PE is_transpose into one bf16 PSUM bank + two 2x-mode
                # DVE copies (halves, so z ch0 can start after the first)
                tp = pz_pool.tile([P, ICB], bf, tag="pz", name="tptail")
                for sub in range(NSUB):
                    nc.tensor.matmul(
                        tp[:, sub * P:(sub + 1) * P],
                        onst[sub][:],
                        ident[:],
                        is_transpose=True,
                        start=(sub == 0),
                        stop=(sub == NSUB - 1),
                        skip_group_check=True,
                    )
                nc.vector.tensor_copy(
                    ot[p][:, icb * ICB:icb * ICB + 512], tp[:, 0:512])
                nc.vector.tensor_copy(
                    ot[p][:, icb * ICB + 512:(icb + 1) * ICB], tp[:, 512:ICB])

        for r, (icb, p) in enumerate(rounds):
            # load this round's background work (deadline-ordered)
            if r == 0:
                bg += [
                    lambda: proj_v_pair(0, pz_pool),
                    lambda: proj_v_pair(2, pz_pool),
                    lambda: proj_v_pair(4, pz_pool),
                    lambda: proj_v_pair(6, pz_pool),
                    lambda: proj_v_pair(8, pz_pool),
                    lambda: proj_qk_half(0, wk, bk_sb, kt_sb, 1, 0, pz_pool),
                    lambda: proj_qk_half(0, wk, bk_sb, kt_sb, 1, 1, pz_pool),
                    lambda: proj_v_pair(10, pz_pool),
                    lambda: proj_v_pair(12, pz_pool),
                    lambda: proj_qk_half(1, wk, bk_sb, kt_sb, 0, 0, pz_pool,
                                         bias_eng="act"),
                    lambda: proj_qk_half(1, wq, bq_sb, qt, 0, 0, pz_pool,
                                         bias_eng="act"),
                    lambda: proj_v_pair(14, pz_pool),
                    lambda: proj_qk_half(1, wk, bk_sb, kt_sb, 0, 1, pz_pool,
                                         bias_eng="act"),
                    lambda: proj_qk_half(1, wq, bq_sb, qt, 0, 1, pz_pool,
                                         bias_eng="act"),
                ]
            elif r == 1:
                bg += [
                    lambda: proj_qk_half(1, wk, bk_sb, kt_sb, 1, 0, pz_pool,
                                         bias_eng="act"),
                    lambda: proj_qk_half(1, wk, bk_sb, kt_sb, 1, 1, pz_pool,
                                         bias_eng="act"),
                    lambda: proj_qk_half(0, wq, bq_sb, qt, 1, 0, pz_pool,
                                         bias_eng="act"),
                    lambda: proj_qk_half(0, wq, bq_sb, qt, 1, 1, pz_pool,
                                         bias_eng="act"),
                    lambda: proj_qk_half(1, wq, bq_sb, qt, 1, 0, pz_pool,
                                         bias_eng="act"),
                    lambda: proj_qk_half(1, wq, bq_sb, qt, 1, 1, pz_pool,
                                         bias_eng="act"),
                ]
            elif r == 2:
                bg += [None] * 4
                bg += [lambda ft=ft, ch=ch: z_chunk(
                            0, ft, ch, pz_pool,
                            copy_eng=("act" if ft % 2 == 0 else "dve"))
                       for ft in range(F // P) for ch in range(2)]

            if r == 0:
                pts = {(0, 0): pt000, (0, 1): pt001,
                       (1, 0): pt010, (1, 1): pt011}
            else:
                pts = {}
                for h in range(2):
                    for half in range(2):
                        sp = emit_S_half(p, icb, 0, h, half)
                        ptile = pt_pool.tile([P, 512], bf, tag="pt", name="pt")
                        emit_exp(ptile[:], sp[:], exp_eng(r, 0, h))
                        pts[(h, half)] = ptile
            # normalize + transposes of the PREVIOUS round, emitted after
            # jt0's exps so the exp engines prioritize feeding PV(jt0)
            if pending_norm is not None:
                do_normalize(*pending_norm, last=False)
                pending_norm = None
            ob = [po_pool.tile([P, 512], f32, tag="po", name=f"ob{i}")
                  for i in range(3)]
            def emit_PV(jt, pts_jt):
                for h in range(2):
                    hl = 2 * p + h
                    for sub in range(NSUB):
                        ptile = pts_jt[(h, sub // 4)]
                        # start=True zeroes the whole 2KB psum bank row, so
                        # only the first matmul into each ob bank may set it
                        first_in_bank = (sub == 0) or (sub == 7 and h == 0)
                        nc.tensor.matmul(
                            ob_slice(ob, h, sub),
                            ptile[:, (sub % 4) * P:(sub % 4 + 1) * P],
                            v_sb[jt][:, hl * (D + 1):(hl + 1) * (D + 1)],
                            start=(jt == 0 and first_in_bank),
                            stop=(jt == NT - 1),
                            skip_group_check=True,
                        )

            # PV lags one jt behind S/exp emission so exp-engine backlogs
            # (normalize bursts at round boundaries) never stall PE
            pts_prev = None
            for jt in range(NT):
                nxt_pts = None
                if jt + 1 < NT:
                    nxt_s = {}
                    nxt_s[(0, 0)] = emit_S_half(p, icb, jt + 1, 0, 0)
                    nxt_s[(0, 1)] = emit_S_half(p, icb, jt + 1, 0, 1)
                    # slot 1: only pop when backlog exceeds remaining jts
                    if bg and len(bg) > (NT - jt):
                        item = bg.pop(0)
                        if item is not None:
                            item()
                    nxt_s[(1, 0)] = emit_S_half(p, icb, jt + 1, 1, 0)
                    nxt_s[(1, 1)] = emit_S_half(p, icb, jt + 1, 1, 1)
                    nxt_pts = {}
                    for key, sp in nxt_s.items():
                        ptile = pt_pool.tile([P, 512], bf, tag="pt", name="pt")
                        emit_exp(ptile[:], sp[:], exp_eng(r, jt + 1, key[0]))
                        nxt_pts[key] = ptile
                if bg:
                    item = bg.pop(0)
                    if item is not None:
                        item()
                if jt >= 1:
                    emit_PV(jt - 1, pts_prev)
                pts_prev, pts = pts, nxt_pts
            emit_PV(NT - 1, pts_prev)

            pending_norm = (ob, p, icb)

        # ---- final normalize + tail: icb=1 z chunks -------------------------
        do_normalize(*pending_norm, last=True)
        # ch0 chunks first (they only need the first half of r3's O^T copy);
        # all 8 psum banks (ps_s 4 + po 3 + pz 1) so no slot-recycle waits
        z_order = [(0, 0), (1, 0), (2, 0), (3, 0), (0, 1), (1, 1), (2, 1), (3, 1)]
        z_pools = [ps_s, ps_s, ps_s, ps_s, po_pool, po_pool, po_pool, pz_pool]
        for i, (ft, ch) in enumerate(z_order):
            z_chunk(1, ft, ch, z_pools[i],
                    copy_eng=("act" if (ft + ch) % 2 == 0 else "dve"),
                    dma_eng=("act", None, "act", None)[ft])

    nc.compile()
    return nc


def _get_nc():
    if "nc" not in _cache:
        _cache["nc"] = build()
    return _cache["nc"]


def make_in_maps(x, Wq, bq, Wk, bk, Wv, bv, Wo, bo):
    """Host-side sharding: per-core input dict for core c = 2*b + hg."""
    bfnp = _bf_np()
    in_maps = []
    for c in range(8):
        b, hg = divmod(c, 2)
        cs = slice(hg * DH, (hg + 1) * DH)
        wo_s = np.ascontiguousarray(Wo[cs, :])
        in_maps.append({
            "xtb": np.ascontiguousarray(np.asarray(x[b]).T.astype(bfnp)),
            "wkqv": np.ascontiguousarray(
                np.concatenate([Wk[:, cs], Wq[:, cs], Wv[:, cs]], axis=1)
                .astype(bfnp)),
            "wob": np.ascontiguousarray(wo_s.astype(bfnp)),
            "bias4": np.ascontiguousarray(np.concatenate([
                np.asarray(bk[cs]).reshape(NPAIR, P).T,
                np.asarray(bq[cs]).reshape(NPAIR, P).T,
            ], axis=1).astype(np.float32)),
            "ident": np.eye(P, dtype=bfnp),
        })
    return in_maps


def kernel(x, Wq, bq, Wk, bk, Wv, bv, Wo, bo):
    from concourse.bass_utils import run_bass_kernel_spmd

    x = np.asarray(x, dtype=np.float32)
    args = [np.asarray(a, dtype=np.float32) for a in (Wq, bq, Wk, bk, Wv, bv, Wo, bo)]
    Wv_, bv_, Wo_, bo_ = args[4], args[5], args[6], args[7]
    nc = _get_nc()
    in_maps = make_in_maps(x, *args)
    res = run_bass_kernel_spmd(nc, in_maps, list(range(8)))
    zbias = (bv_ @ Wo_ + bo_).astype(np.float32)
    out = np.empty((B, N, F), dtype=np.float32)
    for b in range(B):
        zt0 = np.asarray(res.results[2 * b]["zt"], dtype=np.float32)
        zt1 = np.asarray(res.results[2 * b + 1]["zt"], dtype=np.float32)
        out[b] = (zt0 + zt1).T + zbias
    return out


# revision 22
# speedup vs baseline: 1.0460x; 1.0209x over previous
"""Multi-head self-attention TRN2 Bass kernel.

Problem: x[4,2048,512], 8 heads of d=64, scale 1/sqrt(512) (full feature dim).

Sharding: 8 cores = (batch b in 0..3) x (head-group hg in 0..1). Each core
handles one batch element and 4 heads (256 of the 512 features), computing a
partial output projection z_partial = attn_heads @ Wo[hg rows].  The host
sums the two partials per batch and adds (bv @ Wo + bo).

Engine split (PE is the bottleneck; exp is split across ACT and DVE):
  - S^T tiles [128 j, 1024 i] on PE (f32r, two heads packed via
    tile_position), PV accumulation on PE with a ones-column rowsum.
  - exp: head 0 tiles via ScalarE AF.Exp; head 1 tiles via a custom DVE op
    EXP_POLY16_ANT computing ((x*a2+a1)*x+a0)^16 (deg-2 minimax of
    exp(x*SCALE/16), then 4 squarings; one 8-stage DVE instruction).
  - O_norm [128 i, 128 d] -> O^T transposes via DMA xbar transpose
    (dma_start_transpose), straight into the ot[k] staging layout.
  - z chunks: PE matmul -> ACT copy -> DMA out (biases folded to host).
  - V-ones memsets on GPSIMD (SBUF only); V copies + normalize split DVE/ACT.

Scheduling: per jt, S(jt+1) is emitted before PV(jt); background FIFO items
(projections, z chunks) pop once per jt through the single spare PSUM bank.
"""

import sys
import os

sys.path.insert(0, "/opt/trn_rl_repo")

import numpy as np

B, N, F = 4, 2048, 512
H, D = 8, 64
P = 128
DH = 256   # features per core (4 heads)
NPAIR = 2  # head pairs per core
KT = F // P          # 4 k-tiles over input features
ICB = 1024           # i-chunk per round
NICB = N // ICB      # 2
NT = N // P          # 16 j-tiles
NSUB = ICB // P      # 8 i-subtiles per chunk
SCALE = 1.0 / float(np.float32(F) ** 0.5)

# deg-2 minimax of exp(y) on |y| <= 0.208, y = s*SCALE/16; exp = p^16.
# Fit in fit_poly.py: rel err 5.7e-4 -> 9.2e-3 after ^16 (worst case |S|=75).
EXPC = (1.0000015, 0.0027809313, 3.828529e-06)  # a0, a1*k, a2*k^2 folded

_cache = {}


def _bf_np():
    import ml_dtypes

    return np.dtype(ml_dtypes.bfloat16)


def _exp_op():
    """Register (once) and return the custom DVE poly-exp op."""
    if "exp_op" in _cache:
        return _cache["exp_op"]
    from concourse.dve_spec import Spec, Src0, C0, C1, C2, sq
    from concourse import dve_ops

    def ref(in0, in1, c0, c1, c2):
        x = in0.astype(np.float32)
        p = (x * c2 + c1) * x + c0
        for _ in range(4):
            p = p * p
        return p

    for op in dve_ops.OPS:
        if op.name == "EXP_POLY16_ANT":
            _cache["exp_op"] = op
            return op
    op = dve_ops.DveOp(
        "EXP_POLY16_ANT",
        Spec(body=sq(sq(sq(sq((Src0 * C2 + C1) * Src0 + C0)))), reference=ref),
        subdim=False,
        uops_sha={"v3": "571513505c6c4e44", "v4": "f56cfa9f44bcd3e4"},
    )
    dve_ops.OPS.append(op)
    dve_ops.CUSTOM_DVE_SPECS[op.name] = op.spec
    dve_ops._SUB_OPCODE_FOR_NAME[op.name] = max(
        dve_ops._SUB_OPCODE_FOR_NAME.values()) + 1
    _cache["exp_op"] = op
    return op


def build():
    """Build + bass-compile the per-core program."""
    import concourse.tile as tile
    from concourse import bacc, mybir
    from contextlib import ExitStack

    f32 = mybir.dt.float32
    f32r = mybir.dt.float32r
    bf = mybir.dt.bfloat16
    AF = mybir.ActivationFunctionType
    EXP_OP = _exp_op()

    n = N
    nc = bacc.Bacc("TRN2", target_bir_lowering=False, debug=False)

    xt_d = nc.dram_tensor("xtb", [F, n], bf, kind="ExternalInput").ap()
    wkqv_d = nc.dram_tensor("wkqv", [F, 3 * DH], bf, kind="ExternalInput").ap()
    wo_d = nc.dram_tensor("wob", [DH, F], bf, kind="ExternalInput").ap()
    # packed biases: cols 0-1 bk, 2-3 bq
    bias_d = nc.dram_tensor("bias4", [P, 4], f32, kind="ExternalInput").ap()
    id_d = nc.dram_tensor("ident", [P, P], bf, kind="ExternalInput").ap()
    zt_d = nc.dram_tensor("zt", [F, n], bf, kind="ExternalOutput").ap()

    def dve_exp(dst, src):
        nc.vector._custom_dve(EXP_OP, out=dst, in0=src,
                              s0=EXPC[0], s1=EXPC[1], imm2=EXPC[2])

    with tile.TileContext(nc) as tc, ExitStack() as ctx:
        const = ctx.enter_context(tc.tile_pool(name="const", bufs=1))
        pt_pool = ctx.enter_context(tc.tile_pool(name="pt", bufs=14))
        rc_pool = ctx.enter_context(tc.tile_pool(name="rc", bufs=8))
        on_pool = ctx.enter_context(tc.tile_pool(name="on", bufs=16))
        zs_pool = ctx.enter_context(tc.tile_pool(name="zs", bufs=4))
        ps_s = ctx.enter_context(tc.tile_pool(name="ps_s", bufs=4, space="PSUM"))
        po_pool = ctx.enter_context(tc.tile_pool(name="po", bufs=3, space="PSUM"))
        pz_pool = ctx.enter_context(tc.tile_pool(name="pz", bufs=1, space="PSUM"))

        # ---- DMA loads ------------------------------------------------------
        xt = [const.tile([P, n], bf, tag=f"xt{k}", name=f"xt{k}") for k in range(KT)]
        wkqv = [const.tile([P, 3 * DH], bf, tag=f"wkqv{k}", name=f"wkqv{k}")
                for k in range(KT)]
        wk = [wkqv[k][:, 0:DH] for k in range(KT)]
        wq = [wkqv[k][:, DH:2 * DH] for k in range(KT)]
        wv = [wkqv[k][:, 2 * DH:3 * DH] for k in range(KT)]
        bias_sb = const.tile([P, 4], f32, tag="bias4", name="bias_sb")
        bk_sb = bias_sb[:, 0:NPAIR]
        bq_sb = bias_sb[:, NPAIR:2 * NPAIR]
        # warm tiles first: a dummy matmul starts the PE p-state ramp at t~1us
        # so the prologue projections run at full clock; the exp warms the
        # ScalarE table while DMAs stream in
        warm = const.tile([1, 1], f32, tag="warm", name="warm")
        nc.gpsimd.memset(warm[:], 0.0)
        warmmm = pz_pool.tile([P, 512], f32, tag="pz", name="warmmm")
        nc.tensor.matmul(warmmm[0:1, 0:1], warm[:], warm[:],
                         start=True, stop=True)
        nc.scalar.activation(warm[:], warm[:], AF.Exp)

        # first wave: weights + the first 512 cols of each k-tile (enough for
        # the q/k half-0 projections and j-tiles 0-3) to minimize the
        # DMA-serialized latency to the first S tile
        nc.sync.dma_start(wkqv[0][:], wkqv_d[0:P, :])
        nc.sync.dma_start(xt[0][:, 0:512], xt_d[0:P, 0:512])
        nc.sync.dma_start(bias_sb[:], bias_d[:])
        for k in range(1, KT):
            nc.sync.dma_start(wkqv[k][:], wkqv_d[k * P:(k + 1) * P, :])
            nc.sync.dma_start(xt[k][:, 0:512], xt_d[k * P:(k + 1) * P, 0:512])
        for k in range(KT):
            nc.sync.dma_start(xt[k][:, 512:ICB], xt_d[k * P:(k + 1) * P, 512:ICB])
        for k in range(KT):
            nc.sync.dma_start(xt[k][:, 1024:n], xt_d[k * P:(k + 1) * P, 1024:n])
        wo = [const.tile([P, F], bf, tag=f"wo{k}", name=f"wo{k}") for k in range(DH // P)]
        for k in range(DH // P):
            nc.sync.dma_start(wo[k][:], wo_d[k * P:(k + 1) * P, :])
        ident = const.tile([P, P], bf, tag="ident", name="ident")
        nc.sync.dma_start(ident[:], id_d[:])

        # persistent activations
        qt = [const.tile([P, n], f32r, tag=f"qt{p}", name=f"qt{p}") for p in range(NPAIR)]
        kt_sb = [const.tile([P, n], f32r, tag=f"kt{p}", name=f"ktsb{p}") for p in range(NPAIR)]
        # V per j-tile: [128, 260] bf16, head hl at cols [65*hl, 65*hl+64),
        # ones at col 65*hl+64 (accumulates softmax row-sums in PV).
        v_sb = [const.tile([P, 4 * (D + 1)], bf, tag=f"v{j}", name=f"v{j}")
                for j in range(NT)]
        for j in range(NT):
            nc.gpsimd.memset(v_sb[j][:], 1.0)
        # O^T staging for the z projection: [128 d, n] bf16 per k-tile (pair)
        ot = [const.tile([P, n], bf, tag=f"ot{p}", name=f"ot{p}") for p in range(NPAIR)]

        def v4(ap):
            return ap.rearrange("p (h c) -> p h c", h=4)

        # ---- projections -----------------------------------------------------
        def proj_qk_half(p, w_t, b_sb, dst, ib, half, pool, bias_eng="vector"):
            """One 512-wide half of a Q/K projection chunk."""
            ps = pool.tile([P, 512], f32, tag="pz", name="pjh") if pool is pz_pool \
                else pool.tile([P, 512], f32, tag="st", name="pjs")
            psl = ps[:, 0:512]
            isl = slice(ib * ICB + half * 512, ib * ICB + (half + 1) * 512)
            for k in range(KT):
                nc.tensor.matmul(
                    psl,
                    w_t[k][:, p * P:(p + 1) * P],
                    xt[k][:, isl],
                    start=(k == 0),
                    stop=(k == KT - 1),
                )
            if bias_eng == "act":
                nc.scalar.activation(dst[p][:, isl], psl, AF.Identity,
                                     bias=b_sb[:, p:p + 1])
            else:
                nc.vector.tensor_scalar_add(dst[p][:, isl], psl, b_sb[:, p:p + 1])

        def proj_v_pair(j0, pool):
            """V for j-tiles j0, j0+1 sharing one pz bank (two 256-col slices)."""
            ps = pool.tile([P, 512], f32, tag="pz", name="pvh") if pool is pz_pool \
                else pool.tile([P, 512], f32, tag="st", name="pvs")
            for m, j in enumerate((j0, j0 + 1)):
                psl = ps[:, m * DH:(m + 1) * DH]
                for k in range(KT):
                    nc.tensor.matmul(
                        psl,
                        xt[k][:, j * P:(j + 1) * P],
                        wv[k][:],
                        start=(k == 0 and m == 0),
                        stop=(k == KT - 1 and m == 1),
                        skip_group_check=True,
                    )
            for m, j in enumerate((j0, j0 + 1)):
                psl = ps[:, m * DH:(m + 1) * DH]
                nc.vector.tensor_copy(v4(v_sb[j][:])[:, :, 0:D], v4(psl))

        zsb_by_ft = {}

        def z_chunk(icb, ft, ch, pool, copy_eng="act", dma_eng=None):
            """z^T[ft*128:(ft+1)*128, 512-chunk ch of icb]; both chunks of an
            (icb, ft) share one [128,1024] bf16 staging tile and one DMA."""
            tag = {id(pz_pool): "pz", id(po_pool): "po"}.get(id(pool), "st")
            zp = pool.tile([P, 512], f32, tag=tag, name="zp")
            zpl = zp[:, 0:512]
            isl = slice(icb * ICB + ch * 512, icb * ICB + (ch + 1) * 512)
            for k in range(DH // P):
                nc.tensor.matmul(
                    zpl,
                    wo[k][:, ft * P:(ft + 1) * P],
                    ot[k][:, isl],
                    start=(k == 0),
                    stop=(k == DH // P - 1),
                )
            zsb = zs_pool.tile([P, 512], bf, tag="zt", name="zsb")
            if copy_eng == "act":
                nc.scalar.activation(zsb[:], zpl, AF.Copy)
            else:
                nc.vector.tensor_copy(zsb[:], zpl)
            eng = {None: nc.sync, "act": nc.scalar}[dma_eng]
            eng.dma_start(zt_d[ft * P:(ft + 1) * P, isl], zsb[:])

        # ---- attention rounds ------------------------------------------------
        def ob_slice(ob, h, sub):
            """PSUM accumulator slice [128, 65] for (head h, i-subtile sub)."""
            if sub < 7:
                t = ob[h]
                c0 = sub * (D + 1)
            else:
                t = ob[2]
                c0 = h * (D + 1)
            return t[:, c0:c0 + D + 1]

        def emit_S_half(p, icb, jt, h, half):
            sps = ps_s.tile([P, 512], f32, tag="st", name="sps")
            hp = slice(D * h, D * (h + 1))
            isl = slice(icb * ICB + half * 512, icb * ICB + (half + 1) * 512)
            nc.tensor.matmul(
                sps[:, 0:512],
                kt_sb[p][hp, jt * P:(jt + 1) * P],
                qt[p][hp, isl],
                start=True,
                stop=True,
                tile_position=(D * h, 0),
            )
            return sps

        def emit_exp(dst, src, eng):
            if eng == "act":
                nc.scalar.activation(dst, src, AF.Exp, scale=SCALE)
            else:
                dve_exp(dst, src)

        # exp routing: head 0 -> ACT, head 1 -> DVE; jt0 fully on ACT so the
        # boundary normalize (DVE) never delays PV(jt0)
        def exp_eng(r, jt, h):
            return "act" if (h == 0 or jt == 0) else "dve"

        # prologue: K/Q first chunks + jt0's four S halves + the FIRST EXPS.
        proj_qk_half(0, wq, bq_sb, qt, 0, 0, ps_s, bias_eng="act")
        proj_qk_half(0, wk, bk_sb, kt_sb, 0, 0, ps_s)
        s000 = emit_S_half(0, 0, 0, 0, 0)
        pt000 = pt_pool.tile([P, 512], bf, tag="pt", name="pt000")
        nc.scalar.activation(pt000[:], s000[:], AF.Exp, scale=SCALE)
        proj_qk_half(0, wq, bq_sb, qt, 0, 1, ps_s, bias_eng="act")
        s001 = emit_S_half(0, 0, 0, 0, 1)
        proj_qk_half(0, wk, bk_sb, kt_sb, 0, 1, pz_pool)
        pt001 = pt_pool.tile([P, 512], bf, tag="pt", name="pt001")
        nc.scalar.activation(pt001[:], s001[:], AF.Exp, scale=SCALE)
        s010 = emit_S_half(0, 0, 0, 1, 0)
        s011 = emit_S_half(0, 0, 0, 1, 1)
        pt010 = pt_pool.tile([P, 512], bf, tag="pt", name="pt010")
        dve_exp(pt010[:], s010[:])
        pt011 = pt_pool.tile([P, 512], bf, tag="pt", name="pt011")
        dve_exp(pt011[:], s011[:])

        rounds = [(0, 0), (0, 1), (1, 0), (1, 1)]
        bg = []          # background FIFO: fns emitting pz-chained work
        pending_norm = None

        def transp_subs(p, icb, onst, subs):
            """DMA xbar transposes: onst[sub] [128 i,128 d] -> ot[p] blocks."""
            for sub in subs:
                dst = ot[p][:, icb * ICB + sub * P: icb * ICB + (sub + 1) * P]
                nc.sync.dma_start_transpose(dst, onst[sub][:])

        def do_normalize(ob, p, icb, last):
            """Normalize the finished round's O accumulators -> onst bf16,
            then stage O^T into ot (DMA xbar mid-kernel, PE+DVE at the tail).
            Muls are batched (stride-0 broadcast of the rowsum reciprocals)
            so only ~6 DVE ops land at the round boundary."""
            onst_t = on_pool.tile([P, NSUB * P], bf, tag="on", name="onst")
            onst = [onst_t[:, s * P:(s + 1) * P] for s in range(NSUB)]
            for h in range(2):
                # batched reciprocals: subs 0-6 rowsums (col 64 of each
                # 65-col slice in bank ob[h]) in one strided op, sub 7 single
                rc8 = rc_pool.tile([P, 8], f32, tag="rc8", name="rc8")
                in7 = ob[h][:, 0:7 * (D + 1)].rearrange(
                    "p (s c) -> p s c", c=D + 1)[:, :, D:D + 1]
                out7 = rc8[:, 0:7].rearrange("p (s c) -> p s c", c=1)
                nc.vector.reciprocal(out7, in7)
                nc.vector.reciprocal(
                    rc8[:, 7:8], ob[2][:, h * (D + 1) + D: h * (D + 1) + D + 1])
                src7 = ob[h][:, 0:7 * (D + 1)].rearrange(
                    "p (s c) -> p s c", c=D + 1)[:, :, 0:D]
                dst7 = onst_t[:].rearrange(
                    "p (s hh c) -> p s hh c", hh=2, c=D)[:, 0:7, h, :]
                rb7 = rc8[:, 0:7].rearrange(
                    "p (s c) -> p s c", c=1).broadcast_to([P, 7, D])
                nc.vector.tensor_tensor(dst7, src7, rb7, mybir.AluOpType.mult)
                nc.vector.tensor_scalar_mul(
                    onst_t[:, 7 * P + h * D:7 * P + (h + 1) * D],
                    ob[2][:, h * (D + 1):h * (D + 1) + D], rc8[:, 7:8])
            if not last:
                # O^T via DMA xbar transpose (latency hidden; DMA is idle)
                transp_subs(p, icb, onst, range(NSUB))
            else:
                # tail: PE is_transpose into one bf16 PSUM bank + two 2x-mode
                # DVE copies (halves, so z ch0 can start after the first)
                tp = pz_pool.tile([P, ICB], bf, tag="pz", name="tptail")
                for sub in range(NSUB):
                    nc.tensor.matmul(
                        tp[:, sub * P:(sub + 1) * P],
                        onst[sub][:],
                        ident[:],
                        is_transpose=True,
                        start=(sub == 0),
                        stop=(sub == NSUB - 1),
                        skip_group_check=True,
                    )
                nc.vector.tensor_copy(
                    ot[p][:, icb * ICB:icb * ICB + 512], tp[:, 0:512])
                nc.vector.tensor_copy(
                    ot[p][:, icb * ICB + 512:(icb + 1) * ICB], tp[:, 512:ICB])

        for r, (icb, p) in enumerate(rounds):
            # load this round's background work (deadline-ordered)
            if r == 0:
                bg += [
                    lambda: proj_v_pair(0, pz_pool),
                    lambda: proj_v_pair(2, pz_pool),
                    lambda: proj_v_pair(4, pz_pool),
                    lambda: proj_v_pair(6, pz_pool),
                    lambda: proj_v_pair(8, pz_pool),
                    lambda: proj_qk_half(0, wk, bk_sb, kt_sb, 1, 0, pz_pool),
                    lambda: proj_qk_half(0, wk, bk_sb, kt_sb, 1, 1, pz_pool),
                    lambda: proj_v_pair(10, pz_pool),
                    lambda: proj_v_pair(12, pz_pool),
                    lambda: proj_qk_half(1, wk, bk_sb, kt_sb, 0, 0, pz_pool,
                                         bias_eng="act"),
                    lambda: proj_qk_half(1, wq, bq_sb, qt, 0, 0, pz_pool,
                                         bias_eng="act"),
                    lambda: proj_v_pair(14, pz_pool),
                    lambda: proj_qk_half(1, wk, bk_sb, kt_sb, 0, 1, pz_pool,
                                         bias_eng="act"),
                    lambda: proj_qk_half(1, wq, bq_sb, qt, 0, 1, pz_pool,
                                         bias_eng="act"),
                ]
            elif r == 1:
                bg += [
                    lambda: proj_qk_half(1, wk, bk_sb, kt_sb, 1, 0, pz_pool,
                                         bias_eng="act"),
                    lambda: proj_qk_half(1, wk, bk_sb, kt_sb, 1, 1, pz_pool,
                                         bias_eng="act"),
                    lambda: proj_qk_half(0, wq, bq_sb, qt, 1, 0, pz_pool,
                                         bias_eng="act"),
                    lambda: proj_qk_half(0, wq, bq_sb, qt, 1, 1, pz_pool,
                                         bias_eng="act"),
                    lambda: proj_qk_half(1, wq, bq_sb, qt, 1, 0, pz_pool,
                                         bias_eng="act"),
                    lambda: proj_qk_half(1, wq, bq_sb, qt, 1, 1, pz_pool,
                                         bias_eng="act"),
                ]
            elif r == 2:
                bg += [None] * 6
                bg += [lambda ft=ft, ch=ch: z_chunk(
                            0, ft, ch, pz_pool,
                            copy_eng=("act" if ft % 2 == 0 else "dve"))
                       for ft in range(F // P) for ch in range(2)]

            if r == 0:
                pts = {(0, 0): pt000, (0, 1): pt001,
                       (1, 0): pt010, (1, 1): pt011}
            else:
                pts = {}
                for h in range(2):
                    for half in range(2):
                        sp = emit_S_half(p, icb, 0, h, half)
                        ptile = pt_pool.tile([P, 512], bf, tag="pt", name="pt")
                        emit_exp(ptile[:], sp[:], exp_eng(r, 0, h))
                        pts[(h, half)] = ptile
            # normalize + transposes of the PREVIOUS round, emitted after
            # jt0's exps so the exp engines prioritize feeding PV(jt0)
            if pending_norm is not None:
                do_normalize(*pending_norm, last=False)
                pending_norm = None
            ob = [po_pool.tile([P, 512], f32, tag="po", name=f"ob{i}")
                  for i in range(3)]
            def emit_PV(jt, pts_jt):
                for h in range(2):
                    hl = 2 * p + h
                    for sub in range(NSUB):
                        ptile = pts_jt[(h, sub // 4)]
                        # start=True zeroes the whole 2KB psum bank row, so
                        # only the first matmul into each ob bank may set it
                        first_in_bank = (sub == 0) or (sub == 7 and h == 0)
                        nc.tensor.matmul(
                            ob_slice(ob, h, sub),
                            ptile[:, (sub % 4) * P:(sub % 4 + 1) * P],
                            v_sb[jt][:, hl * (D + 1):(hl + 1) * (D + 1)],
                            start=(jt == 0 and first_in_bank),
                            stop=(jt == NT - 1),
                            skip_group_check=True,
                        )

            # PV lags one jt behind S/exp emission so exp-engine backlogs
            # (normalize bursts at round boundaries) never stall PE
            pts_prev = None
            for jt in range(NT):
                nxt_pts = None
                if jt + 1 < NT:
                    nxt_s = {}
                    nxt_s[(0, 0)] = emit_S_half(p, icb, jt + 1, 0, 0)
                    nxt_s[(0, 1)] = emit_S_half(p, icb, jt + 1, 0, 1)
                    # slot 1: only pop when backlog exceeds remaining jts
                    if bg and len(bg) > (NT - jt):
                        item = bg.pop(0)
                        if item is not None:
                            item()
                    nxt_s[(1, 0)] = emit_S_half(p, icb, jt + 1, 1, 0)
                    nxt_s[(1, 1)] = emit_S_half(p, icb, jt + 1, 1, 1)
                    nxt_pts = {}
                    for key, sp in nxt_s.items():
                        ptile = pt_pool.tile([P, 512], bf, tag="pt", name="pt")
                        emit_exp(ptile[:], sp[:], exp_eng(r, jt + 1, key[0]))
                        nxt_pts[key] = ptile
                if bg:
                    item = bg.pop(0)
                    if item is not None:
                        item()
                if jt >= 1:
                    emit_PV(jt - 1, pts_prev)
                pts_prev, pts = pts, nxt_pts
            emit_PV(NT - 1, pts_prev)

            pending_norm = (ob, p, icb)

        # ---- final normalize + tail: icb=1 z chunks -------------------------
        do_normalize(*pending_norm, last=True)
        # ch0 chunks first (they only need the first half of r3's O^T copy);
        # all 8 psum banks (ps_s 4 + po 3 + pz 1) so no slot-recycle waits.
        # Copies land in one [128, 4x512] staging tile per ch-group; each
        # group goes out in a single wide DMA (one HWDGE pass, 512 descs).
        zg = [zs_pool.tile([P, 4 * 512], bf, tag="zt", name=f"zg{c}")
              for c in range(2)]
        z_order = [(0, 0), (1, 0), (2, 0), (3, 0), (0, 1), (1, 1), (2, 1), (3, 1)]
        z_pools = [ps_s, ps_s, ps_s, ps_s, po_pool, po_pool, po_pool, pz_pool]
        for i, (ft, ch) in enumerate(z_order):
            zp = z_pools[i].tile(
                [P, 512], f32,
                tag={id(pz_pool): "pz", id(po_pool): "po"}.get(id(z_pools[i]), "st"),
                name="zp")
            isl = slice(ICB + ch * 512, ICB + (ch + 1) * 512)
            for k in range(DH // P):
                nc.tensor.matmul(
                    zp[:, 0:512],
                    wo[k][:, ft * P:(ft + 1) * P],
                    ot[k][:, isl],
                    start=(k == 0),
                    stop=(k == DH // P - 1),
                )
            dst = zg[ch][:, ft * 512:(ft + 1) * 512]
            if (ft + ch) % 2 == 0:
                nc.scalar.activation(dst, zp[:, 0:512], AF.Copy)
            else:
                nc.vector.tensor_copy(dst, zp[:, 0:512])
            if i == 3 or i == 7:
                out_ap = zt_d[0:512, isl].rearrange("(f p) i -> p f i", p=P)
                in_ap = zg[ch][:].rearrange("p (f i) -> p f i", f=4)
                (nc.scalar if ch == 0 else nc.sync).dma_start(out_ap, in_ap)

    nc.compile()
    return nc


def _get_nc():
    if "nc" not in _cache:
        _cache["nc"] = build()
    return _cache["nc"]


def make_in_maps(x, Wq, bq, Wk, bk, Wv, bv, Wo, bo):
    """Host-side sharding: per-core input dict for core c = 2*b + hg."""
    bfnp = _bf_np()
    in_maps = []
    for c in range(8):
        b, hg = divmod(c, 2)
        cs = slice(hg * DH, (hg + 1) * DH)
        wo_s = np.ascontiguousarray(Wo[cs, :])
        in_maps.append({
            "xtb": np.ascontiguousarray(np.asarray(x[b]).T.astype(bfnp)),
            "wkqv": np.ascontiguousarray(
                np.concatenate([Wk[:, cs], Wq[:, cs], Wv[:, cs]], axis=1)
                .astype(bfnp)),
            "wob": np.ascontiguousarray(wo_s.astype(bfnp)),
            "bias4": np.ascontiguousarray(np.concatenate([
                np.asarray(bk[cs]).reshape(NPAIR, P).T,
                np.asarray(bq[cs]).reshape(NPAIR, P).T,
            ], axis=1).astype(np.float32)),
            "ident": np.eye(P, dtype=bfnp),
        })
    return in_maps


def kernel(x, Wq, bq, Wk, bk, Wv, bv, Wo, bo):
    from concourse.bass_utils import run_bass_kernel_spmd

    x = np.asarray(x, dtype=np.float32)
    args = [np.asarray(a, dtype=np.float32) for a in (Wq, bq, Wk, bk, Wv, bv, Wo, bo)]
    Wv_, bv_, Wo_, bo_ = args[4], args[5], args[6], args[7]
    nc = _get_nc()
    in_maps = make_in_maps(x, *args)
    res = run_bass_kernel_spmd(nc, in_maps, list(range(8)))
    zbias = (bv_ @ Wo_ + bo_).astype(np.float32)
    out = np.empty((B, N, F), dtype=np.float32)
    for b in range(B):
        zt0 = np.asarray(res.results[2 * b]["zt"], dtype=np.float32)
        zt1 = np.asarray(res.results[2 * b + 1]["zt"], dtype=np.float32)
        out[b] = (zt0 + zt1).T + zbias
    return out


# revision 23
# speedup vs baseline: 1.0481x; 1.0019x over previous
"""Multi-head self-attention TRN2 Bass kernel.

Problem: x[4,2048,512], 8 heads of d=64, scale 1/sqrt(512) (full feature dim).

Sharding: 8 cores = (batch b in 0..3) x (head-group hg in 0..1). Each core
handles one batch element and 4 heads (256 of the 512 features), computing a
partial output projection z_partial = attn_heads @ Wo[hg rows].  The host
sums the two partials per batch and adds (bv @ Wo + bo).

Engine split (PE is the bottleneck; exp is split across ACT and DVE):
  - S^T tiles [128 j, 1024 i] on PE (f32r, two heads packed via
    tile_position), PV accumulation on PE with a ones-column rowsum.
  - exp: head 0 tiles via ScalarE AF.Exp; head 1 tiles via a custom DVE op
    EXP_POLY16_ANT computing ((x*a2+a1)*x+a0)^16 (deg-2 minimax of
    exp(x*SCALE/16), then 4 squarings; one 8-stage DVE instruction).
  - O_norm [128 i, 128 d] -> O^T transposes via DMA xbar transpose
    (dma_start_transpose), straight into the ot[k] staging layout.
  - z chunks: PE matmul -> ACT copy -> DMA out (biases folded to host).
  - V-ones memsets on GPSIMD (SBUF only); V copies + normalize split DVE/ACT.

Scheduling: per jt, S(jt+1) is emitted before PV(jt); background FIFO items
(projections, z chunks) pop once per jt through the single spare PSUM bank.
"""

import sys
import os

sys.path.insert(0, "/opt/trn_rl_repo")

import numpy as np

B, N, F = 4, 2048, 512
H, D = 8, 64
P = 128
DH = 256   # features per core (4 heads)
NPAIR = 2  # head pairs per core
KT = F // P          # 4 k-tiles over input features
ICB = 1024           # i-chunk per round
NICB = N // ICB      # 2
NT = N // P          # 16 j-tiles
NSUB = ICB // P      # 8 i-subtiles per chunk
SCALE = 1.0 / float(np.float32(F) ** 0.5)

# deg-2 minimax of exp(y) on |y| <= 0.208, y = s*SCALE/16; exp = p^16.
# Fit in fit_poly.py: rel err 5.7e-4 -> 9.2e-3 after ^16 (worst case |S|=75).
EXPC = (1.0000015, 0.0027809313, 3.828529e-06)  # a0, a1*k, a2*k^2 folded

_cache = {}


def _bf_np():
    import ml_dtypes

    return np.dtype(ml_dtypes.bfloat16)


def _exp_op():
    """Register (once) and return the custom DVE poly-exp op."""
    if "exp_op" in _cache:
        return _cache["exp_op"]
    from concourse.dve_spec import Spec, Src0, C0, C1, C2, sq
    from concourse import dve_ops

    def ref(in0, in1, c0, c1, c2):
        x = in0.astype(np.float32)
        p = (x * c2 + c1) * x + c0
        for _ in range(4):
            p = p * p
        return p

    for op in dve_ops.OPS:
        if op.name == "EXP_POLY16_ANT":
            _cache["exp_op"] = op
            return op
    op = dve_ops.DveOp(
        "EXP_POLY16_ANT",
        Spec(body=sq(sq(sq(sq((Src0 * C2 + C1) * Src0 + C0)))), reference=ref),
        subdim=False,
        uops_sha={"v3": "571513505c6c4e44", "v4": "f56cfa9f44bcd3e4"},
    )
    dve_ops.OPS.append(op)
    dve_ops.CUSTOM_DVE_SPECS[op.name] = op.spec
    dve_ops._SUB_OPCODE_FOR_NAME[op.name] = max(
        dve_ops._SUB_OPCODE_FOR_NAME.values()) + 1
    _cache["exp_op"] = op
    return op


def build():
    """Build + bass-compile the per-core program."""
    import concourse.tile as tile
    from concourse import bacc, mybir
    from contextlib import ExitStack

    f32 = mybir.dt.float32
    f32r = mybir.dt.float32r
    bf = mybir.dt.bfloat16
    AF = mybir.ActivationFunctionType
    EXP_OP = _exp_op()

    n = N
    nc = bacc.Bacc("TRN2", target_bir_lowering=False, debug=False)

    xt_d = nc.dram_tensor("xtb", [F, n], bf, kind="ExternalInput").ap()
    wkqv_d = nc.dram_tensor("wkqv", [F, 3 * DH], bf, kind="ExternalInput").ap()
    wo_d = nc.dram_tensor("wob", [DH, F], bf, kind="ExternalInput").ap()
    # packed biases: cols 0-1 bk, 2-3 bq
    bias_d = nc.dram_tensor("bias4", [P, 4], f32, kind="ExternalInput").ap()
    id_d = nc.dram_tensor("ident", [P, P], bf, kind="ExternalInput").ap()
    zt_d = nc.dram_tensor("zt", [F, n], bf, kind="ExternalOutput").ap()

    def dve_exp(dst, src):
        nc.vector._custom_dve(EXP_OP, out=dst, in0=src,
                              s0=EXPC[0], s1=EXPC[1], imm2=EXPC[2])

    with tile.TileContext(nc) as tc, ExitStack() as ctx:
        const = ctx.enter_context(tc.tile_pool(name="const", bufs=1))
        pt_pool = ctx.enter_context(tc.tile_pool(name="pt", bufs=18))
        rc_pool = ctx.enter_context(tc.tile_pool(name="rc", bufs=8))
        on_pool = ctx.enter_context(tc.tile_pool(name="on", bufs=16))
        zs_pool = ctx.enter_context(tc.tile_pool(name="zs", bufs=4))
        ps_s = ctx.enter_context(tc.tile_pool(name="ps_s", bufs=4, space="PSUM"))
        po_pool = ctx.enter_context(tc.tile_pool(name="po", bufs=3, space="PSUM"))
        pz_pool = ctx.enter_context(tc.tile_pool(name="pz", bufs=1, space="PSUM"))

        # ---- DMA loads ------------------------------------------------------
        xt = [const.tile([P, n], bf, tag=f"xt{k}", name=f"xt{k}") for k in range(KT)]
        wkqv = [const.tile([P, 3 * DH], bf, tag=f"wkqv{k}", name=f"wkqv{k}")
                for k in range(KT)]
        wk = [wkqv[k][:, 0:DH] for k in range(KT)]
        wq = [wkqv[k][:, DH:2 * DH] for k in range(KT)]
        wv = [wkqv[k][:, 2 * DH:3 * DH] for k in range(KT)]
        bias_sb = const.tile([P, 4], f32, tag="bias4", name="bias_sb")
        bk_sb = bias_sb[:, 0:NPAIR]
        bq_sb = bias_sb[:, NPAIR:2 * NPAIR]
        # warm tiles first: a dummy matmul starts the PE p-state ramp at t~1us
        # so the prologue projections run at full clock; the exp warms the
        # ScalarE table while DMAs stream in
        warm = const.tile([1, 1], f32, tag="warm", name="warm")
        nc.gpsimd.memset(warm[:], 0.0)
        warmmm = pz_pool.tile([P, 512], f32, tag="pz", name="warmmm")
        nc.tensor.matmul(warmmm[0:1, 0:1], warm[:], warm[:],
                         start=True, stop=True)
        nc.scalar.activation(warm[:], warm[:], AF.Exp)

        # first wave: weights + the first 512 cols of each k-tile (enough for
        # the q/k half-0 projections and j-tiles 0-3) to minimize the
        # DMA-serialized latency to the first S tile
        nc.sync.dma_start(wkqv[0][:], wkqv_d[0:P, :])
        nc.sync.dma_start(xt[0][:, 0:512], xt_d[0:P, 0:512])
        nc.sync.dma_start(bias_sb[:], bias_d[:])
        for k in range(1, KT):
            nc.sync.dma_start(wkqv[k][:], wkqv_d[k * P:(k + 1) * P, :])
            nc.sync.dma_start(xt[k][:, 0:512], xt_d[k * P:(k + 1) * P, 0:512])
        for k in range(KT):
            nc.sync.dma_start(xt[k][:, 512:ICB], xt_d[k * P:(k + 1) * P, 512:ICB])
        for k in range(KT):
            nc.sync.dma_start(xt[k][:, 1024:n], xt_d[k * P:(k + 1) * P, 1024:n])
        wo = [const.tile([P, F], bf, tag=f"wo{k}", name=f"wo{k}") for k in range(DH // P)]
        for k in range(DH // P):
            nc.sync.dma_start(wo[k][:], wo_d[k * P:(k + 1) * P, :])
        ident = const.tile([P, P], bf, tag="ident", name="ident")
        nc.sync.dma_start(ident[:], id_d[:])

        # persistent activations
        qt = [const.tile([P, n], f32r, tag=f"qt{p}", name=f"qt{p}") for p in range(NPAIR)]
        kt_sb = [const.tile([P, n], f32r, tag=f"kt{p}", name=f"ktsb{p}") for p in range(NPAIR)]
        # V per j-tile: [128, 260] bf16, head hl at cols [65*hl, 65*hl+64),
        # ones at col 65*hl+64 (accumulates softmax row-sums in PV).
        v_sb = [const.tile([P, 4 * (D + 1)], bf, tag=f"v{j}", name=f"v{j}")
                for j in range(NT)]
        for j in range(NT):
            nc.gpsimd.memset(v_sb[j][:], 1.0)
        # O^T staging for the z projection: [128 d, n] bf16 per k-tile (pair)
        ot = [const.tile([P, n], bf, tag=f"ot{p}", name=f"ot{p}") for p in range(NPAIR)]

        def v4(ap):
            return ap.rearrange("p (h c) -> p h c", h=4)

        # ---- projections -----------------------------------------------------
        def proj_qk_half(p, w_t, b_sb, dst, ib, half, pool, bias_eng="vector"):
            """One 512-wide half of a Q/K projection chunk."""
            ps = pool.tile([P, 512], f32, tag="pz", name="pjh") if pool is pz_pool \
                else pool.tile([P, 512], f32, tag="st", name="pjs")
            psl = ps[:, 0:512]
            isl = slice(ib * ICB + half * 512, ib * ICB + (half + 1) * 512)
            for k in range(KT):
                nc.tensor.matmul(
                    psl,
                    w_t[k][:, p * P:(p + 1) * P],
                    xt[k][:, isl],
                    start=(k == 0),
                    stop=(k == KT - 1),
                )
            if bias_eng == "act":
                nc.scalar.activation(dst[p][:, isl], psl, AF.Identity,
                                     bias=b_sb[:, p:p + 1])
            else:
                nc.vector.tensor_scalar_add(dst[p][:, isl], psl, b_sb[:, p:p + 1])

        def proj_v_pair(j0, pool):
            """V for j-tiles j0, j0+1 sharing one pz bank (two 256-col slices)."""
            ps = pool.tile([P, 512], f32, tag="pz", name="pvh") if pool is pz_pool \
                else pool.tile([P, 512], f32, tag="st", name="pvs")
            for m, j in enumerate((j0, j0 + 1)):
                psl = ps[:, m * DH:(m + 1) * DH]
                for k in range(KT):
                    nc.tensor.matmul(
                        psl,
                        xt[k][:, j * P:(j + 1) * P],
                        wv[k][:],
                        start=(k == 0 and m == 0),
                        stop=(k == KT - 1 and m == 1),
                        skip_group_check=True,
                    )
            for m, j in enumerate((j0, j0 + 1)):
                psl = ps[:, m * DH:(m + 1) * DH]
                nc.vector.tensor_copy(v4(v_sb[j][:])[:, :, 0:D], v4(psl))

        zsb_by_ft = {}

        def z_chunk(icb, ft, ch, pool, copy_eng="act", dma_eng=None):
            """z^T[ft*128:(ft+1)*128, 512-chunk ch of icb]; both chunks of an
            (icb, ft) share one [128,1024] bf16 staging tile and one DMA."""
            tag = {id(pz_pool): "pz", id(po_pool): "po"}.get(id(pool), "st")
            zp = pool.tile([P, 512], f32, tag=tag, name="zp")
            zpl = zp[:, 0:512]
            isl = slice(icb * ICB + ch * 512, icb * ICB + (ch + 1) * 512)
            for k in range(DH // P):
                nc.tensor.matmul(
                    zpl,
                    wo[k][:, ft * P:(ft + 1) * P],
                    ot[k][:, isl],
                    start=(k == 0),
                    stop=(k == DH // P - 1),
                )
            zsb = zs_pool.tile([P, 512], bf, tag="zt", name="zsb")
            if copy_eng == "act":
                nc.scalar.activation(zsb[:], zpl, AF.Copy)
            else:
                nc.vector.tensor_copy(zsb[:], zpl)
            eng = {None: nc.sync, "act": nc.scalar}[dma_eng]
            eng.dma_start(zt_d[ft * P:(ft + 1) * P, isl], zsb[:])

        # ---- attention rounds ------------------------------------------------
        def ob_slice(ob, h, sub):
            """PSUM accumulator slice [128, 65] for (head h, i-subtile sub)."""
            if sub < 7:
                t = ob[h]
                c0 = sub * (D + 1)
            else:
                t = ob[2]
                c0 = h * (D + 1)
            return t[:, c0:c0 + D + 1]

        def emit_S_half(p, icb, jt, h, half):
            sps = ps_s.tile([P, 512], f32, tag="st", name="sps")
            hp = slice(D * h, D * (h + 1))
            isl = slice(icb * ICB + half * 512, icb * ICB + (half + 1) * 512)
            nc.tensor.matmul(
                sps[:, 0:512],
                kt_sb[p][hp, jt * P:(jt + 1) * P],
                qt[p][hp, isl],
                start=True,
                stop=True,
                tile_position=(D * h, 0),
            )
            return sps

        def emit_exp(dst, src, eng):
            if eng == "act":
                nc.scalar.activation(dst, src, AF.Exp, scale=SCALE)
            else:
                dve_exp(dst, src)

        # exp routing: head 0 -> ACT, head 1 -> DVE; jt0 fully on ACT so the
        # boundary normalize (DVE) never delays PV(jt0)
        def exp_eng(r, jt, h):
            return "act" if (h == 0 or jt == 0) else "dve"

        # prologue: K/Q first chunks + jt0's four S halves + the FIRST EXPS.
        proj_qk_half(0, wq, bq_sb, qt, 0, 0, ps_s, bias_eng="act")
        proj_qk_half(0, wk, bk_sb, kt_sb, 0, 0, ps_s)
        s000 = emit_S_half(0, 0, 0, 0, 0)
        pt000 = pt_pool.tile([P, 512], bf, tag="pt", name="pt000")
        nc.scalar.activation(pt000[:], s000[:], AF.Exp, scale=SCALE)
        proj_qk_half(0, wq, bq_sb, qt, 0, 1, ps_s, bias_eng="act")
        s001 = emit_S_half(0, 0, 0, 0, 1)
        proj_qk_half(0, wk, bk_sb, kt_sb, 0, 1, pz_pool)
        pt001 = pt_pool.tile([P, 512], bf, tag="pt", name="pt001")
        nc.scalar.activation(pt001[:], s001[:], AF.Exp, scale=SCALE)
        s010 = emit_S_half(0, 0, 0, 1, 0)
        s011 = emit_S_half(0, 0, 0, 1, 1)
        pt010 = pt_pool.tile([P, 512], bf, tag="pt", name="pt010")
        dve_exp(pt010[:], s010[:])
        pt011 = pt_pool.tile([P, 512], bf, tag="pt", name="pt011")
        dve_exp(pt011[:], s011[:])

        rounds = [(0, 0), (0, 1), (1, 0), (1, 1)]
        bg = []          # background FIFO: fns emitting pz-chained work
        pending_norm = None

        def transp_subs(p, icb, onst, subs):
            """DMA xbar transposes: onst[sub] [128 i,128 d] -> ot[p] blocks."""
            for sub in subs:
                dst = ot[p][:, icb * ICB + sub * P: icb * ICB + (sub + 1) * P]
                nc.sync.dma_start_transpose(dst, onst[sub][:])

        def do_normalize(ob, p, icb, last):
            """Normalize the finished round's O accumulators -> onst bf16,
            then stage O^T into ot (DMA xbar mid-kernel, PE+DVE at the tail).
            Muls are batched (stride-0 broadcast of the rowsum reciprocals)
            so only ~6 DVE ops land at the round boundary."""
            onst_t = on_pool.tile([P, NSUB * P], bf, tag="on", name="onst")
            onst = [onst_t[:, s * P:(s + 1) * P] for s in range(NSUB)]
            for h in range(2):
                # batched reciprocals: subs 0-6 rowsums (col 64 of each
                # 65-col slice in bank ob[h]) in one strided op, sub 7 single
                rc8 = rc_pool.tile([P, 8], f32, tag="rc8", name="rc8")
                in7 = ob[h][:, 0:7 * (D + 1)].rearrange(
                    "p (s c) -> p s c", c=D + 1)[:, :, D:D + 1]
                out7 = rc8[:, 0:7].rearrange("p (s c) -> p s c", c=1)
                nc.vector.reciprocal(out7, in7)
                nc.vector.reciprocal(
                    rc8[:, 7:8], ob[2][:, h * (D + 1) + D: h * (D + 1) + D + 1])
                src7 = ob[h][:, 0:7 * (D + 1)].rearrange(
                    "p (s c) -> p s c", c=D + 1)[:, :, 0:D]
                dst7 = onst_t[:].rearrange(
                    "p (s hh c) -> p s hh c", hh=2, c=D)[:, 0:7, h, :]
                rb7 = rc8[:, 0:7].rearrange(
                    "p (s c) -> p s c", c=1).broadcast_to([P, 7, D])
                nc.vector.tensor_tensor(dst7, src7, rb7, mybir.AluOpType.mult)
                nc.vector.tensor_scalar_mul(
                    onst_t[:, 7 * P + h * D:7 * P + (h + 1) * D],
                    ob[2][:, h * (D + 1):h * (D + 1) + D], rc8[:, 7:8])
            if not last:
                # O^T via DMA xbar transpose (latency hidden; DMA is idle)
                transp_subs(p, icb, onst, range(NSUB))
            else:
                # tail: PE is_transpose into one bf16 PSUM bank + two 2x-mode
                # DVE copies (halves, so z ch0 can start after the first)
                tp = pz_pool.tile([P, ICB], bf, tag="pz", name="tptail")
                for sub in range(NSUB):
                    nc.tensor.matmul(
                        tp[:, sub * P:(sub + 1) * P],
                        onst[sub][:],
                        ident[:],
                        is_transpose=True,
                        start=(sub == 0),
                        stop=(sub == NSUB - 1),
                        skip_group_check=True,
                    )
                nc.vector.tensor_copy(
                    ot[p][:, icb * ICB:icb * ICB + 512], tp[:, 0:512])
                nc.vector.tensor_copy(
                    ot[p][:, icb * ICB + 512:(icb + 1) * ICB], tp[:, 512:ICB])

        for r, (icb, p) in enumerate(rounds):
            # load this round's background work (deadline-ordered)
            if r == 0:
                bg += [
                    lambda: proj_v_pair(0, pz_pool),
                    lambda: proj_v_pair(2, pz_pool),
                    lambda: proj_v_pair(4, pz_pool),
                    lambda: proj_v_pair(6, pz_pool),
                    lambda: proj_v_pair(8, pz_pool),
                    lambda: proj_qk_half(0, wk, bk_sb, kt_sb, 1, 0, pz_pool),
                    lambda: proj_qk_half(0, wk, bk_sb, kt_sb, 1, 1, pz_pool),
                    lambda: proj_v_pair(10, pz_pool),
                    lambda: proj_v_pair(12, pz_pool),
                    lambda: proj_qk_half(1, wk, bk_sb, kt_sb, 0, 0, pz_pool,
                                         bias_eng="act"),
                    lambda: proj_qk_half(1, wq, bq_sb, qt, 0, 0, pz_pool,
                                         bias_eng="act"),
                    lambda: proj_v_pair(14, pz_pool),
                    lambda: proj_qk_half(1, wk, bk_sb, kt_sb, 0, 1, pz_pool,
                                         bias_eng="act"),
                    lambda: proj_qk_half(1, wq, bq_sb, qt, 0, 1, pz_pool,
                                         bias_eng="act"),
                ]
            elif r == 1:
                bg += [
                    lambda: proj_qk_half(1, wk, bk_sb, kt_sb, 1, 0, pz_pool,
                                         bias_eng="act"),
                    lambda: proj_qk_half(1, wk, bk_sb, kt_sb, 1, 1, pz_pool,
                                         bias_eng="act"),
                    lambda: proj_qk_half(0, wq, bq_sb, qt, 1, 0, pz_pool,
                                         bias_eng="act"),
                    lambda: proj_qk_half(0, wq, bq_sb, qt, 1, 1, pz_pool,
                                         bias_eng="act"),
                    lambda: proj_qk_half(1, wq, bq_sb, qt, 1, 0, pz_pool,
                                         bias_eng="act"),
                    lambda: proj_qk_half(1, wq, bq_sb, qt, 1, 1, pz_pool,
                                         bias_eng="act"),
                ]
            elif r == 2:
                bg += [None] * 6
                bg += [lambda ft=ft, ch=ch: z_chunk(
                            0, ft, ch, pz_pool,
                            copy_eng=("act" if ft % 2 == 0 else "dve"))
                       for ft in range(F // P) for ch in range(2)]

            if r == 0:
                pts = {(0, 0): pt000, (0, 1): pt001,
                       (1, 0): pt010, (1, 1): pt011}
            else:
                pts = {}
                for h in range(2):
                    for half in range(2):
                        sp = emit_S_half(p, icb, 0, h, half)
                        ptile = pt_pool.tile([P, 512], bf, tag="pt", name="pt")
                        emit_exp(ptile[:], sp[:], exp_eng(r, 0, h))
                        pts[(h, half)] = ptile
            # normalize + transposes of the PREVIOUS round, emitted after
            # jt0's exps so the exp engines prioritize feeding PV(jt0)
            if pending_norm is not None:
                do_normalize(*pending_norm, last=False)
                pending_norm = None
            ob = [po_pool.tile([P, 512], f32, tag="po", name=f"ob{i}")
                  for i in range(3)]
            def emit_PV(jt, pts_jt):
                for h in range(2):
                    hl = 2 * p + h
                    for sub in range(NSUB):
                        ptile = pts_jt[(h, sub // 4)]
                        # start=True zeroes the whole 2KB psum bank row, so
                        # only the first matmul into each ob bank may set it
                        first_in_bank = (sub == 0) or (sub == 7 and h == 0)
                        nc.tensor.matmul(
                            ob_slice(ob, h, sub),
                            ptile[:, (sub % 4) * P:(sub % 4 + 1) * P],
                            v_sb[jt][:, hl * (D + 1):(hl + 1) * (D + 1)],
                            start=(jt == 0 and first_in_bank),
                            stop=(jt == NT - 1),
                            skip_group_check=True,
                        )

            # PV lags LAG jts behind S/exp emission so exp-engine backlogs
            # (normalize bursts at round boundaries) never stall PE
            LAG = 2
            pts_q = [(0, pts)]
            for jt in range(NT):
                if jt + 1 < NT:
                    nxt_s = {}
                    nxt_s[(0, 0)] = emit_S_half(p, icb, jt + 1, 0, 0)
                    nxt_s[(0, 1)] = emit_S_half(p, icb, jt + 1, 0, 1)
                    # slot 1: only pop when backlog exceeds remaining jts
                    if bg and len(bg) > (NT - jt):
                        item = bg.pop(0)
                        if item is not None:
                            item()
                    nxt_s[(1, 0)] = emit_S_half(p, icb, jt + 1, 1, 0)
                    nxt_s[(1, 1)] = emit_S_half(p, icb, jt + 1, 1, 1)
                    nxt_pts = {}
                    for key, sp in nxt_s.items():
                        ptile = pt_pool.tile([P, 512], bf, tag="pt", name="pt")
                        emit_exp(ptile[:], sp[:], exp_eng(r, jt + 1, key[0]))
                        nxt_pts[key] = ptile
                    pts_q.append((jt + 1, nxt_pts))
                if bg:
                    item = bg.pop(0)
                    if item is not None:
                        item()
                if len(pts_q) > LAG:
                    emit_PV(*pts_q.pop(0))
            while pts_q:
                emit_PV(*pts_q.pop(0))

            pending_norm = (ob, p, icb)

        # ---- final normalize + tail: icb=1 z chunks -------------------------
        do_normalize(*pending_norm, last=True)
        # ch0 chunks first (they only need the first half of r3's O^T copy);
        # all 8 psum banks (ps_s 4 + po 3 + pz 1) so no slot-recycle waits.
        # Copies land in one [128, 4x512] staging tile per ch-group; each
        # group goes out in a single wide DMA (one HWDGE pass, 512 descs).
        zg = [zs_pool.tile([P, 4 * 512], bf, tag="zt", name=f"zg{c}")
              for c in range(2)]
        z_order = [(0, 0), (1, 0), (2, 0), (3, 0), (0, 1), (1, 1), (2, 1), (3, 1)]
        z_pools = [ps_s, ps_s, ps_s, ps_s, po_pool, po_pool, po_pool, pz_pool]
        for i, (ft, ch) in enumerate(z_order):
            zp = z_pools[i].tile(
                [P, 512], f32,
                tag={id(pz_pool): "pz", id(po_pool): "po"}.get(id(z_pools[i]), "st"),
                name="zp")
            isl = slice(ICB + ch * 512, ICB + (ch + 1) * 512)
            for k in range(DH // P):
                nc.tensor.matmul(
                    zp[:, 0:512],
                    wo[k][:, ft * P:(ft + 1) * P],
                    ot[k][:, isl],
                    start=(k == 0),
                    stop=(k == DH // P - 1),
                )
            dst = zg[ch][:, ft * 512:(ft + 1) * 512]
            if (ft + ch) % 2 == 0:
                nc.scalar.activation(dst, zp[:, 0:512], AF.Copy)
            else:
                nc.vector.tensor_copy(dst, zp[:, 0:512])
            if i == 3 or i == 7:
                out_ap = zt_d[0:512, isl].rearrange("(f p) i -> p f i", p=P)
                in_ap = zg[ch][:].rearrange("p (f i) -> p f i", f=4)
                (nc.scalar if ch == 0 else nc.sync).dma_start(out_ap, in_ap)

    nc.compile()
    return nc


def _get_nc():
    if "nc" not in _cache:
        _cache["nc"] = build()
    return _cache["nc"]


def make_in_maps(x, Wq, bq, Wk, bk, Wv, bv, Wo, bo):
    """Host-side sharding: per-core input dict for core c = 2*b + hg."""
    bfnp = _bf_np()
    in_maps = []
    for c in range(8):
        b, hg = divmod(c, 2)
        cs = slice(hg * DH, (hg + 1) * DH)
        wo_s = np.ascontiguousarray(Wo[cs, :])
        in_maps.append({
            "xtb": np.ascontiguousarray(np.asarray(x[b]).T.astype(bfnp)),
            "wkqv": np.ascontiguousarray(
                np.concatenate([Wk[:, cs], Wq[:, cs], Wv[:, cs]], axis=1)
                .astype(bfnp)),
            "wob": np.ascontiguousarray(wo_s.astype(bfnp)),
            "bias4": np.ascontiguousarray(np.concatenate([
                np.asarray(bk[cs]).reshape(NPAIR, P).T,
                np.asarray(bq[cs]).reshape(NPAIR, P).T,
            ], axis=1).astype(np.float32)),
            "ident": np.eye(P, dtype=bfnp),
        })
    return in_maps


def kernel(x, Wq, bq, Wk, bk, Wv, bv, Wo, bo):
    from concourse.bass_utils import run_bass_kernel_spmd

    x = np.asarray(x, dtype=np.float32)
    args = [np.asarray(a, dtype=np.float32) for a in (Wq, bq, Wk, bk, Wv, bv, Wo, bo)]
    Wv_, bv_, Wo_, bo_ = args[4], args[5], args[6], args[7]
    nc = _get_nc()
    in_maps = make_in_maps(x, *args)
    res = run_bass_kernel_spmd(nc, in_maps, list(range(8)))
    zbias = (bv_ @ Wo_ + bo_).astype(np.float32)
    out = np.empty((B, N, F), dtype=np.float32)
    for b in range(B):
        zt0 = np.asarray(res.results[2 * b]["zt"], dtype=np.float32)
        zt1 = np.asarray(res.results[2 * b + 1]["zt"], dtype=np.float32)
        out[b] = (zt0 + zt1).T + zbias
    return out
